# revision 1
# baseline (speedup 1.0000x reference)
"""Trainium2 Bass kernel for a multi-head attention block (B=4, S=2048, D=1024, H=16).

Sharding over 8 NeuronCores: core c handles batch b=c//2 and head-group
hg=c%2 (8 of 16 heads). Each core computes its heads' QKV projections,
causal attention, and a partial output projection (row-sharded Wo); the
2-way reduction per batch (the "all-reduce after w_o") happens on host at
gather time, along with the bo bias (bo/2 added on each device).

Device dataflow (all bf16 matmuls, fp32 PSUM accumulate; activations kept
transposed, feature-on-partition): one loop over q-blocks of 512; each
iteration computes that block's kT/v/qT projections, then causal
attention per head-pair: scoresT[k,q] blocks for both heads land in one
2-bank PSUM tile (the K=64 score matmuls run concurrently on disjoint PE
row groups), one merged exp on ScalarE produces bf16 probs, diagonal
blocks are column-trimmed to the causal region and masked by a binary
triangle multiply post-exp, PV matmuls accumulate [65, 512] per head
(row 64 = softmax denominator via a ones column in v), then
reciprocal + partition-broadcast + multiply normalize into paired
[128, 512] tiles (odd head shifted via SBUF-to-SBUF DMA) feeding a
K=128 Wo projection. The projection for each block is emitted after the
next block's QKV so its normalize latency hides under QKV matmuls.
"""

import numpy as np
import ml_dtypes
from contextlib import ExitStack

import concourse.bass as bass
import concourse.tile as tile
from concourse import bacc, mybir
from concourse.bass_utils import run_bass_kernel_spmd
from concourse.alu_op_type import AluOpType

F32 = mybir.dt.float32
F32R = mybir.dt.float32r
BF16 = mybir.dt.bfloat16

S = 2048          # sequence length
D = 1024          # model dim
NH = 8            # heads per core
DKH = 64          # head dim
NHP = 4           # head pairs per core
SB = 512          # seq block (q block)
NSB = S // SB     # 4
KC = 128          # k chunk
NDIN = D // 128   # 8 input-dim chunks
VW = DKH + 1      # 65: v columns per head incl. ones column
NEG = -1.0e9


def _r(ap):
    return ap.bitcast(F32R)


def build_program():
    nc = bacc.Bacc("TRN2", target_bir_lowering=False, debug=False, num_devices=8)

    xq = nc.dram_tensor("xq_t", [D, S], BF16, kind="ExternalInput").ap()
    xk = nc.dram_tensor("xk_t", [D, S], BF16, kind="ExternalInput").ap()
    xv = nc.dram_tensor("xv_t", [D, S], BF16, kind="ExternalInput").ap()
    wq = nc.dram_tensor("wq_t", [D, 512], BF16, kind="ExternalInput").ap()
    wkv = nc.dram_tensor("wkv_t", [D, 1024], BF16, kind="ExternalInput").ap()
    wo = nc.dram_tensor("wo_t", [512, D], BF16, kind="ExternalInput").ap()
    bqs = nc.dram_tensor("bq_s", [512, 1], F32, kind="ExternalInput").ap()
    bks = nc.dram_tensor("bk_s", [512, 1], F32, kind="ExternalInput").ap()
    bvr = nc.dram_tensor("bv_r", [1, 512], F32, kind="ExternalInput").ap()
    bor = nc.dram_tensor("bo_r", [1, D], F32, kind="ExternalInput").ap()
    y = nc.dram_tensor("y", [S, D], F32, kind="ExternalOutput").ap()

    with tile.TileContext(nc) as tc, ExitStack() as ctx:
        const = ctx.enter_context(tc.tile_pool(name="const", bufs=1))
        wpool = ctx.enter_context(tc.tile_pool(name="w", bufs=1))
        acts = ctx.enter_context(tc.tile_pool(name="acts", bufs=1))
        qpool = ctx.enter_context(tc.tile_pool(name="qpool", bufs=2))
        xs = ctx.enter_context(tc.tile_pool(name="xs", bufs=16))
        probs = ctx.enter_context(tc.tile_pool(name="probs", bufs=6))
        atp = ctx.enter_context(tc.tile_pool(name="at", bufs=3))
        nrm = ctx.enter_context(tc.tile_pool(name="nrm", bufs=6))
        ysb = ctx.enter_context(tc.tile_pool(name="ysb", bufs=4))
        mmps = ctx.enter_context(tc.tile_pool(name="mmps", bufs=2, space="PSUM"))
        scps = ctx.enter_context(tc.tile_pool(name="scps", bufs=2, space="PSUM"))
        pvps = ctx.enter_context(tc.tile_pool(name="pvps", bufs=2, space="PSUM"))

        # trigger the gpsimd custom-op library load immediately
        dum1 = const.tile([1, 16], F32, name="dum1", tag="dum1")
        nc.vector.memset(dum1[:], 0.0)
        dum2 = const.tile([8, 16], F32, name="dum2", tag="dum2")
        nc.gpsimd.partition_broadcast(dum2[:], dum1[:])

        # ---- constants ----
        ones = const.tile([128, 256], BF16, name="ones", tag="ones")
        nc.vector.memset(ones[:], 1.0)
        # binary causal triangle, duplicated for both heads: [128, (2, 128)]
        # keep 1.0 where q' - k' >= 0 (within the diagonal 128-wide strip)
        tri = const.tile([128, 256], BF16, name="tri", tag="tri")
        nc.gpsimd.affine_select(
            out=tri[:].rearrange("p (h q) -> p h q", h=2),
            in_=ones[:].rearrange("p (h q) -> p h q", h=2),
            compare_op=AluOpType.is_ge,
            fill=0.0,
            base=0,
            pattern=[[0, 2], [1, 128]],
            channel_multiplier=-1,
        )

        bv_row = const.tile([1, 512], F32, name="bvr", tag="bvr")
        nc.scalar.dma_start(bv_row[:], bvr[:])
        bv_b = const.tile([128, 512], F32, name="bvb", tag="bvb")
        nc.gpsimd.partition_broadcast(bv_b[:], bv_row[:])

        bo_row = const.tile([1, D], F32, name="bor", tag="bor")
        nc.scalar.dma_start(bo_row[:], bor[:])
        bo_b = const.tile([128, D], F32, name="bob", tag="bob")
        nc.gpsimd.partition_broadcast(bo_b[:], bo_row[:])

        bq_t, bk_t = [], []
        for m_i in range(4):
            t = const.tile([128, 1], F32, name=f"bq{m_i}", tag=f"bq{m_i}")
            nc.scalar.dma_start(t[:], bqs[m_i * 128:(m_i + 1) * 128, :])
            bq_t.append(t)
            t = const.tile([128, 1], F32, name=f"bk{m_i}", tag=f"bk{m_i}")
            nc.scalar.dma_start(t[:], bks[m_i * 128:(m_i + 1) * 128, :])
            bk_t.append(t)

        # ---- weights ----
        wqc, wkvc = [], []
        for c in range(NDIN):
            t = wpool.tile([128, 1024], BF16, name=f"wkv{c}", tag=f"wkv{c}")
            nc.scalar.dma_start(t[:], wkv[c * 128:(c + 1) * 128, :])
            wkvc.append(t)
        for c in range(NDIN):
            t = wpool.tile([128, 512], BF16, name=f"wq{c}", tag=f"wq{c}")
            nc.scalar.dma_start(t[:], wq[c * 128:(c + 1) * 128, :])
            wqc.append(t)
        woh2 = []
        for hp in range(NHP):
            t = wpool.tile([128, D], BF16, name=f"wo{hp}", tag=f"wo{hp}")
            nc.scalar.dma_start(t[:], wo[hp * 128:(hp + 1) * 128, :])
            woh2.append(t)

        # ---- resident k/v activations ----
        kT = {}
        for hp in range(NHP):
            for sb in range(NSB):
                kT[(hp, sb)] = acts.tile([128, SB], BF16, name=f"kT{hp}_{sb}",
                                         tag=f"kT{hp}_{sb}")
        vt = [acts.tile([128, NH * VW], BF16, name=f"v{t_i}", tag=f"v{t_i}")
              for t_i in range(S // KC)]

        def emit_proj(p_sb, p_at):
            for m_i in range(4):
                pss = [mmps.tile([128, SB], F32, name="yps", tag="mm") for _ in range(2)]
                for hp in range(NHP):
                    for n_i in range(2):
                        nc.tensor.matmul(
                            pss[n_i][:],
                            p_at[hp][:, m_i * 128:(m_i + 1) * 128],
                            woh2[hp][:, n_i * SB:(n_i + 1) * SB],
                            start=(hp == 0),
                            stop=(hp == NHP - 1),
                        )
                for n_i in range(2):
                    yt = ysb.tile([128, SB], F32, name="yt", tag="y")
                    nc.vector.tensor_tensor(yt[:], pss[n_i][:], bo_b[:, n_i * SB:(n_i + 1) * SB], AluOpType.add)
                    eng = nc.sync if n_i == 0 else nc.gpsimd
                    eng.dma_start(
                        y[p_sb * SB + m_i * 128: p_sb * SB + (m_i + 1) * 128,
                          n_i * SB:(n_i + 1) * SB],
                        yt[:],
                    )

        prev = None
        # ---- main loop over seq blocks ----
        for sb in range(NSB):
            ssl = slice(sb * SB, (sb + 1) * SB)

            # kT block [512, SB]: lhsT = wkT chunk, rhs = xk chunk
            xcs = []
            for c in range(NDIN):
                xc = xs.tile([128, SB], BF16, name="x", tag="x")
                nc.sync.dma_start(xc[:], xk[c * 128:(c + 1) * 128, ssl])
                xcs.append(xc)
            for m_i in range(4):
                ps = mmps.tile([128, SB], F32, name="mm", tag="mm")
                for c in range(NDIN):
                    nc.tensor.matmul(
                        ps[:],
                        wkvc[c][:, m_i * 128:(m_i + 1) * 128],
                        xcs[c][:],
                        start=(c == 0),
                        stop=(c == NDIN - 1),
                    )
                nc.vector.tensor_scalar_add(kT[(m_i, sb)][:], ps[:], bk_t[m_i][:])

            # v block [SB, 512] -> strided into vt tiles (bf16) with ones col
            xcs = []
            for c in range(NDIN):
                xc = xs.tile([128, SB], BF16, name="x", tag="x")
                nc.sync.dma_start(xc[:], xv[c * 128:(c + 1) * 128, ssl])
                xcs.append(xc)
            for m_i in range(4):
                ps2 = mmps.tile([128, SB], F32, name="mm", tag="mm")
                for c in range(NDIN):
                    nc.tensor.matmul(
                        ps2[:],
                        xcs[c][:, m_i * 128:(m_i + 1) * 128],
                        wkvc[c][:, 512:1024],
                        start=(c == 0),
                        stop=(c == NDIN - 1),
                    )
                t = vt[sb * 4 + m_i]
                t3 = t[:].rearrange("p (h c) -> p h c", h=NH)
                nc.vector.tensor_tensor(
                    t3[:, :, 0:DKH],
                    ps2[:].rearrange("p (h c) -> p h c", h=NH),
                    bv_b[:].rearrange("p (h c) -> p h c", h=NH),
                    AluOpType.add,
                )
                nc.vector.memset(t3[:, :, DKH:VW], 1.0)

            # qT block [512, SB]
            xcs = []
            for c in range(NDIN):
                xc = xs.tile([128, SB], BF16, name="x", tag="x")
                nc.sync.dma_start(xc[:], xq[c * 128:(c + 1) * 128, ssl])
                xcs.append(xc)
            qT = [None] * 4
            for m_i in range(4):
                ps = mmps.tile([128, SB], F32, name="mm", tag="mm")
                for c in range(NDIN):
                    nc.tensor.matmul(
                        ps[:],
                        wqc[c][:, m_i * 128:(m_i + 1) * 128],
                        xcs[c][:],
                        start=(c == 0),
                        stop=(c == NDIN - 1),
                    )
                qt = qpool.tile([128, SB], BF16, name=f"qT{m_i}", tag=f"qT{m_i}")
                nc.vector.tensor_scalar_add(qt[:], ps[:], bq_t[m_i][:])
                qT[m_i] = qt

            if prev is not None:
                emit_proj(*prev)

            # ---- attention for this q block ----
            at_tiles = [None] * NHP
            for hp in range(NHP):
                nck = 4 * sb + 4
                pvA = pvps.tile([128, SB], F32, name="pvA", tag="pv")
                pvB = pvps.tile([128, SB], F32, name="pvB", tag="pv")
                for kc in range(nck):
                    kts = kT[(hp, kc // 4)]
                    koff = (kc % 4) * 128
                    r_i = kc - 4 * sb
                    qoff = 128 * r_i if r_i > 0 else 0  # diag chunks: only q >= k start
                    nq = SB - qoff
                    # merged 2-bank scores psum: head A in cols 0:512, B in 512:1024
                    psAB = scps.tile([128, 2 * SB], F32, name="sAB", tag="sc")
                    nc.tensor.matmul(
                        psAB[:, qoff:SB],
                        kts[0:64, koff:koff + 128],
                        qT[hp][0:64, qoff:SB],
                        start=True, stop=True,
                    )
                    nc.tensor.matmul(
                        psAB[:, SB + qoff:2 * SB],
                        kts[64:128, koff:koff + 128],
                        qT[hp][64:128, qoff:SB],
                        start=True, stop=True,
                    )
                    # one exp over both heads' (trimmed) scores
                    pAB = probs.tile([128, 2 * SB], BF16, name="pAB", tag="probs")
                    ps3 = psAB[:].rearrange("p (h q) -> p h q", h=2)[:, :, qoff:SB]
                    pr3 = pAB[:].rearrange("p (h q) -> p h q", h=2)[:, :, qoff:SB]
                    nc.scalar.activation(pr3, ps3, mybir.ActivationFunctionType.Exp, scale=0.125)
                    if r_i >= 0:
                        # zero the strictly-lower triangle of the 128-wide diag strip
                        tr3 = pAB[:].rearrange("p (h q) -> p h q", h=2)[:, :, qoff:qoff + 128]
                        mk3 = tri[:].rearrange("p (h q) -> p h q", h=2)
                        nc.vector.tensor_tensor(tr3, tr3, mk3, AluOpType.mult)
                    vtile = vt[kc]
                    hA, hB = 2 * hp, 2 * hp + 1
                    nc.tensor.matmul(
                        pvA[0:VW, qoff:SB], vtile[:, hA * VW:(hA + 1) * VW],
                        pAB[:, qoff:SB],
                        start=(kc == 0), stop=(kc == nck - 1),
                    )
                    nc.tensor.matmul(
                        pvB[0:VW, qoff:SB], vtile[:, hB * VW:(hB + 1) * VW],
                        pAB[:, SB + qoff:2 * SB],
                        start=(kc == 0), stop=(kc == nck - 1),
                    )
                # copy out of PSUM promptly to free the accumulators
                pvcA = nrm.tile([VW, SB], F32, name="pvcA", tag="pvc")
                nc.vector.tensor_copy(pvcA[:], pvA[0:VW, :])
                pvcB = nrm.tile([VW, SB], F32, name="pvcB", tag="pvc")
                nc.vector.tensor_copy(pvcB[:], pvB[0:VW, :])
                at = atp.tile([128, SB], BF16, name=f"at{hp}", tag=f"at{hp}")
                for h_sub, pvc in ((0, pvcA), (1, pvcB)):
                    rc0 = nrm.tile([1, SB], F32, name="rc0", tag="rc0")
                    nc.sync.dma_start(rc0[:], pvc[64:65, :])
                    rc1 = nrm.tile([1, SB], F32, name="rc1", tag="rc1")
                    nc.vector.reciprocal_approx_fast(rc1[:], rc0[:])
                    rb = nrm.tile([64, SB], F32, name="rb", tag="rb")
                    nc.gpsimd.partition_broadcast(rb[:], rc1[:])
                    if h_sub == 0:
                        nc.vector.tensor_tensor(at[0:64, :], pvc[0:64, :], rb[:], AluOpType.mult)
                    else:
                        ato = atp.tile([64, SB], BF16, name="ato", tag="ato")
                        nc.vector.tensor_tensor(ato[:], pvc[0:64, :], rb[:], AluOpType.mult)
                        nc.sync.dma_start(at[64:128, :], ato[:])
                at_tiles[hp] = at
            prev = (sb, at_tiles)

        emit_proj(*prev)

    nc.compile()
    return nc


_NC = None
_LAST_IN_MAPS = None


def _get_nc():
    global _NC
    if _NC is None:
        _NC = build_program()
    return _NC


def kernel(query, key, value, mask, Wq, bq, Wk, bk, Wv, bv, Wo, bo):
    query = np.asarray(query, np.float32)
    key = np.asarray(key, np.float32)
    value = np.asarray(value, np.float32)
    Wq = np.asarray(Wq, np.float32)
    Wk = np.asarray(Wk, np.float32)
    Wv = np.asarray(Wv, np.float32)
    Wo = np.asarray(Wo, np.float32)
    bq = np.asarray(bq, np.float32)
    bk = np.asarray(bk, np.float32)
    bv = np.asarray(bv, np.float32)
    bo = np.asarray(bo, np.float32)

    nc = _get_nc()

    B = query.shape[0]
    bf = ml_dtypes.bfloat16
    xq_t = [np.ascontiguousarray(query[b].T.astype(bf)) for b in range(B)]
    xk_t = [np.ascontiguousarray(key[b].T.astype(bf)) for b in range(B)]
    xv_t = [np.ascontiguousarray(value[b].T.astype(bf)) for b in range(B)]

    in_maps = []
    for c in range(8):
        b, hg = c // 2, c % 2
        sl = slice(hg * 512, (hg + 1) * 512)
        in_maps.append({
            "xq_t": xq_t[b],
            "xk_t": xk_t[b],
            "xv_t": xv_t[b],
            "wq_t": np.ascontiguousarray(Wq[sl, :].T.astype(bf)),
            "wkv_t": np.ascontiguousarray(
                np.concatenate([Wk[sl, :].T, Wv[sl, :].T], axis=1).astype(bf)),
            "wo_t": np.ascontiguousarray(Wo[:, sl].T.astype(bf)),
            "bq_s": np.ascontiguousarray(bq[sl, None]),
            "bk_s": np.ascontiguousarray(bk[sl, None]),
            "bv_r": np.ascontiguousarray(bv[None, sl]),
            "bo_r": np.ascontiguousarray(bo[None, :] * 0.5),
        })

    global _LAST_IN_MAPS
    _LAST_IN_MAPS = in_maps
    res = run_bass_kernel_spmd(nc, in_maps, core_ids=list(range(8)))
    out = np.empty((B, S, D), np.float32)
    for b in range(B):
        out[b] = res.results[2 * b]["y"] + res.results[2 * b + 1]["y"]
    return out



# revision 5
# speedup vs baseline: 1.0568x; 1.0568x over previous
"""Trainium2 Bass kernel for a multi-head attention block (B=4, S=2048, D=1024, H=16).

Sharding over 8 NeuronCores: core c handles batch b=c//2 and head-group
hg=c%2 (8 of 16 heads). Each core computes its heads' QKV projections,
causal attention, and a partial output projection (row-sharded Wo); the
2-way reduction per batch (the "all-reduce after w_o") happens on host at
gather time, along with the bo bias (bo/2 added on each device).

Device dataflow (all bf16 matmuls, fp32 PSUM accumulate; activations kept
transposed, feature-on-partition). The schedule is software-pipelined so
the PE never idles waiting for softmax:

- Attention for q-block sb runs as a stream of (head-pair, k-chunk)
  "chunks": two K=64 score matmuls on disjoint PE row quadrants (they
  run concurrently), an additive causal-mask matmul into the diag score
  PSUM (lhsT = strict-upper * -1e9, rhs = identity), one merged exp on
  ScalarE for both heads, and two PV matmuls that lag the scores by 2
  chunks so they never wait on the exp.
- The QKV projection matmuls for block sb+1 and the Wo projection for
  block sb-1 are interleaved 1-6 per chunk into the attention stream
  ("fills"), so the exp-paced gaps are filled with useful PE work and
  the PE stays at full DVFS p-state.
- The softmax denominator rides as a ones-column in v (PSUM row 64);
  normalization is reciprocal + partition-broadcast + multiply, off the
  critical path. The final block's projection is split into hp01/hp23
  halves so most of it overlaps the last normalize chain.
"""

import math
from collections import deque

import numpy as np
import ml_dtypes
from contextlib import ExitStack

import concourse.bass as bass
import concourse.tile as tile
from concourse import bacc, mybir
from concourse.bass_utils import run_bass_kernel_spmd
from concourse.alu_op_type import AluOpType

F32 = mybir.dt.float32
BF16 = mybir.dt.bfloat16

S = 2048          # sequence length
D = 1024          # model dim
NH = 8            # heads per core
DKH = 64          # head dim
NHP = 4           # head pairs per core
SB = 512          # seq block (q block)
NSB = S // SB     # 4
KC = 128          # k chunk
NDIN = D // 128   # 8 input-dim chunks
VW = DKH + 1      # 65: v columns per head incl. ones column
NEG = -1.0e9


def build_program():
    nc = bacc.Bacc("TRN2", target_bir_lowering=False, debug=False, num_devices=8)

    xq = nc.dram_tensor("xq_t", [D, S], BF16, kind="ExternalInput").ap()
    xk = nc.dram_tensor("xk_t", [D, S], BF16, kind="ExternalInput").ap()
    xv = nc.dram_tensor("xv_t", [D, S], BF16, kind="ExternalInput").ap()
    wq = nc.dram_tensor("wq_t", [D, 512], BF16, kind="ExternalInput").ap()
    wkv = nc.dram_tensor("wkv_t", [D, 1024], BF16, kind="ExternalInput").ap()
    wo = nc.dram_tensor("wo_t", [512, D], BF16, kind="ExternalInput").ap()
    bqs = nc.dram_tensor("bq_s", [512, 1], F32, kind="ExternalInput").ap()
    bks = nc.dram_tensor("bk_s", [512, 1], F32, kind="ExternalInput").ap()
    bvr = nc.dram_tensor("bv_r", [1, 512], F32, kind="ExternalInput").ap()
    bor = nc.dram_tensor("bo_r", [1, D], F32, kind="ExternalInput").ap()
    msk = nc.dram_tensor("msk", [128, 256], BF16, kind="ExternalInput").ap()
    y = nc.dram_tensor("y", [S, D], F32, kind="ExternalOutput").ap()

    with tile.TileContext(nc) as tc, ExitStack() as ctx:
        const = ctx.enter_context(tc.tile_pool(name="const", bufs=1))
        wpool = ctx.enter_context(tc.tile_pool(name="w", bufs=1))
        acts = ctx.enter_context(tc.tile_pool(name="acts", bufs=1))
        qpool = ctx.enter_context(tc.tile_pool(name="qpool", bufs=2))
        xs = ctx.enter_context(tc.tile_pool(name="xs", bufs=24))
        probs = ctx.enter_context(tc.tile_pool(name="probs", bufs=6))
        atp = ctx.enter_context(tc.tile_pool(name="at", bufs=1))
        atop = ctx.enter_context(tc.tile_pool(name="ato", bufs=2))
        nrm = ctx.enter_context(tc.tile_pool(name="nrm", bufs=4))
        ysb = ctx.enter_context(tc.tile_pool(name="ysb", bufs=4))
        y0p = ctx.enter_context(tc.tile_pool(name="y0p", bufs=1))
        mmps = ctx.enter_context(tc.tile_pool(name="mmps", bufs=2, space="PSUM"))
        scps = ctx.enter_context(tc.tile_pool(name="scps", bufs=2, space="PSUM"))
        pvps = ctx.enter_context(tc.tile_pool(name="pvps", bufs=2, space="PSUM"))

        # trigger the gpsimd custom-op library load immediately
        dum1 = const.tile([1, 16], F32, name="dum1", tag="dum1")
        nc.vector.memset(dum1[:], 0.0)
        dum2 = const.tile([8, 16], F32, name="dum2", tag="dum2")
        nc.gpsimd.partition_broadcast(dum2[:], dum1[:])
        # trigger the Exp act-table load before the first real softmax
        dume = const.tile([1, 16], F32, name="dume", tag="dume")
        nc.scalar.activation(dume[:], dum1[:], mybir.ActivationFunctionType.Exp,
                             scale=1.0)

        # ---- weights / consts, DMA'd in need-order ----
        wkvc = []
        for c in range(NDIN):
            t = wpool.tile([128, 1024], BF16, name=f"wkv{c}", tag=f"wkv{c}")
            nc.scalar.dma_start(t[:], wkv[c * 128:(c + 1) * 128, :])
            wkvc.append(t)

        # causal-mask consts: cols 0:128 = strict-upper * -1e9 (lhsT),
        # cols 128:256 = identity (rhs)
        mskt = const.tile([128, 256], BF16, name="mskt", tag="mskt")
        nc.scalar.dma_start(mskt[:], msk[:])
        triu_neg = mskt[:, 0:128]
        ident = mskt[:, 128:256]

        bq_t, bk_t = [], []
        for m_i in range(4):
            t = const.tile([128, 1], F32, name=f"bq{m_i}", tag=f"bq{m_i}")
            nc.scalar.dma_start(t[:], bqs[m_i * 128:(m_i + 1) * 128, :])
            bq_t.append(t)
            t = const.tile([128, 1], F32, name=f"bk{m_i}", tag=f"bk{m_i}")
            nc.scalar.dma_start(t[:], bks[m_i * 128:(m_i + 1) * 128, :])
            bk_t.append(t)

        wqc = []
        for c in range(NDIN):
            t = wpool.tile([128, 512], BF16, name=f"wq{c}", tag=f"wq{c}")
            nc.scalar.dma_start(t[:], wq[c * 128:(c + 1) * 128, :])
            wqc.append(t)

        bv_row = const.tile([1, 512], F32, name="bvr", tag="bvr")
        nc.scalar.dma_start(bv_row[:], bvr[:])
        bv_b = const.tile([128, 512], F32, name="bvb", tag="bvb")
        nc.gpsimd.partition_broadcast(bv_b[:], bv_row[:])

        bo_row = const.tile([1, D], F32, name="bor", tag="bor")
        nc.scalar.dma_start(bo_row[:], bor[:])
        bo_b = const.tile([128, D], F32, name="bob", tag="bob")
        nc.gpsimd.partition_broadcast(bo_b[:], bo_row[:])

        woh2 = []
        for hp in range(NHP):
            t = wpool.tile([128, D], BF16, name=f"wo{hp}", tag=f"wo{hp}")
            nc.scalar.dma_start(t[:], wo[hp * 128:(hp + 1) * 128, :])
            woh2.append(t)

        # ---- resident k/v activations ----
        kT = {}
        for hp in range(NHP):
            for sb in range(NSB):
                kT[(hp, sb)] = acts.tile([128, SB], BF16, name=f"kT{hp}_{sb}",
                                         tag=f"kT{hp}_{sb}")
        vt = [acts.tile([128, NH * VW], BF16, name=f"v{t_i}", tag=f"v{t_i}")
              for t_i in range(S // KC)]

        qT_of = {}    # sb -> [qt tile per m_i]
        at_of = {}    # sb -> [at tile per hp]
        y0_tiles = {}

        # ---------- fill generators ----------
        class QKVGen:
            """Yields once per PE matmul of block sb's q/k/v projections."""

            def __init__(self, sb):
                self.sb = sb
                self.ssl = slice(sb * SB, (sb + 1) * SB)
                self.xk_tiles = None

            def issue_xk(self):
                self.xk_tiles = []
                for c in range(NDIN):
                    t = xs.tile([128, SB], BF16, name="x", tag="x")
                    nc.sync.dma_start(t[:], xk[c * 128:(c + 1) * 128, self.ssl])
                    self.xk_tiles.append(t)

            def gen(self):
                sb = self.sb
                if self.xk_tiles is None:
                    self.issue_xk()
                for m_i in range(4):
                    ps = mmps.tile([128, SB], F32, name="mm", tag="mm")
                    for c in range(NDIN):
                        nc.tensor.matmul(
                            ps[:],
                            wkvc[c][:, m_i * 128:(m_i + 1) * 128],
                            self.xk_tiles[c][:],
                            start=(c == 0), stop=(c == NDIN - 1),
                        )
                        yield
                    nc.vector.tensor_scalar_add(kT[(m_i, sb)][:], ps[:],
                                                bk_t[m_i][:])
                xv_tiles = []
                for c in range(NDIN):
                    t = xs.tile([128, SB], BF16, name="x", tag="x")
                    nc.sync.dma_start(t[:], xv[c * 128:(c + 1) * 128, self.ssl])
                    xv_tiles.append(t)
                for m_i in range(4):
                    ps2 = mmps.tile([128, SB], F32, name="mm", tag="mm")
                    for c in range(NDIN):
                        nc.tensor.matmul(
                            ps2[:],
                            xv_tiles[c][:, m_i * 128:(m_i + 1) * 128],
                            wkvc[c][:, 512:1024],
                            start=(c == 0), stop=(c == NDIN - 1),
                        )
                        yield
                    t = vt[sb * 4 + m_i]
                    t3 = t[:].rearrange("p (h c) -> p h c", h=NH)
                    nc.vector.tensor_tensor(
                        t3[:, :, 0:DKH],
                        ps2[:].rearrange("p (h c) -> p h c", h=NH),
                        bv_b[:].rearrange("p (h c) -> p h c", h=NH),
                        AluOpType.add,
                    )
                    nc.vector.memset(t3[:, :, DKH:VW], 1.0)
                xq_tiles = []
                for c in range(NDIN):
                    t = xs.tile([128, SB], BF16, name="x", tag="x")
                    nc.sync.dma_start(t[:], xq[c * 128:(c + 1) * 128, self.ssl])
                    xq_tiles.append(t)
                qts = [None] * 4
                for m_i in range(4):
                    ps = mmps.tile([128, SB], F32, name="mm", tag="mm")
                    for c in range(NDIN):
                        nc.tensor.matmul(
                            ps[:],
                            wqc[c][:, m_i * 128:(m_i + 1) * 128],
                            xq_tiles[c][:],
                            start=(c == 0), stop=(c == NDIN - 1),
                        )
                        yield
                    qt = qpool.tile([128, SB], BF16, name=f"qT{m_i}",
                                    tag=f"qT{m_i}")
                    nc.vector.tensor_scalar_add(qt[:], ps[:], bq_t[m_i][:])
                    qts[m_i] = qt
                qT_of[sb] = qts

        def proj_gen(sb):
            """Yields once per PE matmul of block sb's output projection."""
            ats = at_of[sb]
            for m_i in range(4):
                for n_i in range(2):
                    ps = mmps.tile([128, SB], F32, name="mm", tag="mm")
                    for hp in range(NHP):
                        nc.tensor.matmul(
                            ps[:],
                            ats[hp][:, m_i * 128:(m_i + 1) * 128],
                            woh2[hp][:, n_i * SB:(n_i + 1) * SB],
                            start=(hp == 0), stop=(hp == NHP - 1),
                        )
                        yield
                    yt = ysb.tile([128, SB], F32, name="yt", tag="y")
                    nc.vector.tensor_tensor(yt[:], ps[:],
                                            bo_b[:, n_i * SB:(n_i + 1) * SB],
                                            AluOpType.add)
                    eng = nc.sync if n_i == 0 else nc.gpsimd
                    eng.dma_start(
                        y[sb * SB + m_i * 128: sb * SB + (m_i + 1) * 128,
                          n_i * SB:(n_i + 1) * SB],
                        yt[:],
                    )

        def proj3_part1():
            """First half (hp 0,1) of block 3's projection; 16 yields."""
            ats = at_of[3]
            for m_i in range(4):
                for n_i in range(2):
                    ps = mmps.tile([128, SB], F32, name="mm", tag="mm")
                    for hp in (0, 1):
                        nc.tensor.matmul(
                            ps[:],
                            ats[hp][:, m_i * 128:(m_i + 1) * 128],
                            woh2[hp][:, n_i * SB:(n_i + 1) * SB],
                            start=(hp == 0), stop=(hp == 1),
                        )
                        yield
                    y0 = y0p.tile([128, SB], F32, name="y0",
                                  tag=f"y0_{m_i}_{n_i}")
                    nc.vector.tensor_tensor(y0[:], ps[:],
                                            bo_b[:, n_i * SB:(n_i + 1) * SB],
                                            AluOpType.add)
                    y0_tiles[(m_i, n_i)] = y0

        def proj3_part2():
            ats = at_of[3]
            for m_i in range(4):
                for n_i in range(2):
                    ps = mmps.tile([128, SB], F32, name="mm", tag="mm")
                    for hp in (2, 3):
                        nc.tensor.matmul(
                            ps[:],
                            ats[hp][:, m_i * 128:(m_i + 1) * 128],
                            woh2[hp][:, n_i * SB:(n_i + 1) * SB],
                            start=(hp == 2), stop=(hp == 3),
                        )
                    yt = ysb.tile([128, SB], F32, name="yt", tag="y")
                    nc.vector.tensor_tensor(yt[:], ps[:],
                                            y0_tiles[(m_i, n_i)][:],
                                            AluOpType.add)
                    eng = nc.sync if n_i == 0 else nc.gpsimd
                    eng.dma_start(
                        y[3 * SB + m_i * 128: 3 * SB + (m_i + 1) * 128,
                          n_i * SB:(n_i + 1) * SB],
                        yt[:],
                    )

        class Pacer:
            def __init__(self):
                self.gens = deque()
                self.pending = 0
                self.slots_left = 0

            def add(self, gen, n):
                self.gens.append(gen)
                self.pending += n

            def draw(self, k):
                while k > 0 and self.gens:
                    try:
                        next(self.gens[0])
                        self.pending -= 1
                        k -= 1
                    except StopIteration:
                        self.gens.popleft()

            def slot(self):
                if self.slots_left > 0:
                    k = int(self.pending / self.slots_left + 0.5)
                    self.slots_left -= 1
                    self.draw(k)

            def drain(self):
                self.draw(self.pending + 1)
                self.slots_left = 0

        # ---------- attention emitters ----------
        def emit_scores(sb, hp, kc):
            kts = kT[(hp, kc // 4)]
            koff = (kc % 4) * 128
            r_i = kc - 4 * sb
            qoff = 128 * r_i if r_i > 0 else 0
            diag = r_i >= 0
            qts = qT_of[sb]
            psAB = scps.tile([128, 2 * SB], F32, name="sAB", tag="sc")
            nc.tensor.matmul(
                psAB[:, qoff:SB],
                kts[0:64, koff:koff + 128],
                qts[hp][0:64, qoff:SB],
                start=True, stop=not diag,
            )
            nc.tensor.matmul(
                psAB[:, SB + qoff:2 * SB],
                kts[64:128, koff:koff + 128],
                qts[hp][64:128, qoff:SB],
                start=True, stop=not diag,
            )
            if diag:
                # additive causal mask: psum[k, qoff+j] += -1e9 * [j < k]
                nc.tensor.matmul(
                    psAB[:, qoff:qoff + 128], triu_neg, ident,
                    start=False, stop=True, skip_group_check=True,
                )
                nc.tensor.matmul(
                    psAB[:, SB + qoff:SB + qoff + 128], triu_neg, ident,
                    start=False, stop=True, skip_group_check=True,
                )
            pAB = probs.tile([128, 2 * SB], BF16, name="pAB", tag="probs")
            ps3 = psAB[:].rearrange("p (h q) -> p h q", h=2)[:, :, qoff:SB]
            pr3 = pAB[:].rearrange("p (h q) -> p h q", h=2)[:, :, qoff:SB]
            nc.scalar.activation(pr3, ps3, mybir.ActivationFunctionType.Exp,
                                 scale=0.125)
            return (kc, pAB, qoff)

        def emit_pv(hp, item, pvA, pvB, nck):
            kc, pAB, qoff = item
            vtile = vt[kc]
            hA, hB = 2 * hp, 2 * hp + 1
            nc.tensor.matmul(
                pvA[0:VW, qoff:SB], vtile[:, hA * VW:(hA + 1) * VW],
                pAB[:, qoff:SB],
                start=(kc == 0), stop=(kc == nck - 1),
            )
            nc.tensor.matmul(
                pvB[0:VW, qoff:SB], vtile[:, hB * VW:(hB + 1) * VW],
                pAB[:, SB + qoff:2 * SB],
                start=(kc == 0), stop=(kc == nck - 1),
            )

        def emit_normalize(sb, hp, pvA, pvB):
            pvcA = nrm.tile([VW, SB], F32, name="pvcA", tag="pvc")
            nc.vector.tensor_copy(pvcA[:], pvA[0:VW, :])
            pvcB = nrm.tile([VW, SB], F32, name="pvcB", tag="pvc")
            nc.vector.tensor_copy(pvcB[:], pvB[0:VW, :])
            at = atp.tile([128, SB], BF16, name="at", tag=f"at{sb}_{hp}")
            for h_sub, pvc in ((0, pvcA), (1, pvcB)):
                rc0 = nrm.tile([1, SB], F32, name="rc0", tag="rc0")
                nc.sync.dma_start(rc0[:], pvc[64:65, :])
                rc1 = nrm.tile([1, SB], F32, name="rc1", tag="rc1")
                nc.vector.reciprocal_approx_fast(rc1[:], rc0[:])
                rb = nrm.tile([64, SB], F32, name="rb", tag="rb")
                nc.gpsimd.partition_broadcast(rb[:], rc1[:])
                if h_sub == 0:
                    nc.vector.tensor_tensor(at[0:64, :], pvc[0:64, :], rb[:],
                                            AluOpType.mult)
                else:
                    ato = atop.tile([64, SB], BF16, name="ato", tag="ato")
                    nc.vector.tensor_tensor(ato[:], pvc[0:64, :], rb[:],
                                            AluOpType.mult)
                    nc.sync.dma_start(at[64:128, :], ato[:])
            return at

        # ---------- schedule ----------
        gens = {sb: QKVGen(sb) for sb in range(NSB)}
        pacer = Pacer()

        # prologue: QKV(s0) dense; prefetch xk(s1) so sb0's fills don't stall
        gens[0].issue_xk()
        gens[1].issue_xk()
        pacer.add(gens[0].gen(), 96)
        pacer.drain()

        for sb in range(NSB):
            if sb < NSB - 1:
                pacer.add(gens[sb + 1].gen(), 96)
            else:
                for psb in range(3):
                    pacer.add(proj_gen(psb), 32)
            nck = 4 * sb + 4
            pacer.slots_left = NHP * nck
            ats = [None] * NHP
            for hp in range(NHP):
                pvA = pvps.tile([128, SB], F32, name="pvA", tag="pv")
                pvB = pvps.tile([128, SB], F32, name="pvB", tag="pv")
                pend = deque()
                for kc in range(nck):
                    pend.append(emit_scores(sb, hp, kc))
                    if len(pend) == 3:
                        emit_pv(hp, pend.popleft(), pvA, pvB, nck)
                    pacer.slot()
                while pend:
                    emit_pv(hp, pend.popleft(), pvA, pvB, nck)
                    pacer.draw(1)
                ats[hp] = emit_normalize(sb, hp, pvA, pvB)
                if sb == NSB - 1 and hp == 1:
                    at_of[3] = ats  # list mutated in place as hp 2,3 finish
                    pacer.add(proj3_part1(), 16)
            at_of[sb] = ats
            pacer.drain()

        proj3_part2()

    nc.compile()
    return nc


_NC = None
_LAST_IN_MAPS = None


def _get_nc():
    global _NC
    if _NC is None:
        _NC = build_program()
    return _NC


def _build_msk():
    m = np.concatenate(
        [np.triu(np.full((128, 128), NEG, np.float32), 1),
         np.eye(128, dtype=np.float32)],
        axis=1,
    )
    return np.ascontiguousarray(m.astype(ml_dtypes.bfloat16))


def kernel(query, key, value, mask, Wq, bq, Wk, bk, Wv, bv, Wo, bo):
    query = np.asarray(query, np.float32)
    key = np.asarray(key, np.float32)
    value = np.asarray(value, np.float32)
    Wq = np.asarray(Wq, np.float32)
    Wk = np.asarray(Wk, np.float32)
    Wv = np.asarray(Wv, np.float32)
    Wo = np.asarray(Wo, np.float32)
    bq = np.asarray(bq, np.float32)
    bk = np.asarray(bk, np.float32)
    bv = np.asarray(bv, np.float32)
    bo = np.asarray(bo, np.float32)

    nc = _get_nc()

    B = query.shape[0]
    bf = ml_dtypes.bfloat16
    xq_t = [np.ascontiguousarray(query[b].T.astype(bf)) for b in range(B)]
    xk_t = [np.ascontiguousarray(key[b].T.astype(bf)) for b in range(B)]
    xv_t = [np.ascontiguousarray(value[b].T.astype(bf)) for b in range(B)]
    msk_np = _build_msk()

    in_maps = []
    for c in range(8):
        b, hg = c // 2, c % 2
        sl = slice(hg * 512, (hg + 1) * 512)
        in_maps.append({
            "xq_t": xq_t[b],
            "xk_t": xk_t[b],
            "xv_t": xv_t[b],
            "wq_t": np.ascontiguousarray(Wq[sl, :].T.astype(bf)),
            "wkv_t": np.ascontiguousarray(
                np.concatenate([Wk[sl, :].T, Wv[sl, :].T], axis=1).astype(bf)),
            "wo_t": np.ascontiguousarray(Wo[:, sl].T.astype(bf)),
            "bq_s": np.ascontiguousarray(bq[sl, None]),
            "bk_s": np.ascontiguousarray(bk[sl, None]),
            "bv_r": np.ascontiguousarray(bv[None, sl]),
            "bo_r": np.ascontiguousarray(bo[None, :] * 0.5),
            "msk": msk_np,
        })

    global _LAST_IN_MAPS
    _LAST_IN_MAPS = in_maps
    res = run_bass_kernel_spmd(nc, in_maps, core_ids=list(range(8)))
    out = np.empty((B, S, D), np.float32)
    for b in range(B):
        out[b] = res.results[2 * b]["y"] + res.results[2 * b + 1]["y"]
    return out


# revision 9
# speedup vs baseline: 1.0765x; 1.0186x over previous
"""Trainium2 Bass kernel for a multi-head attention block (B=4, S=2048, D=1024, H=16).

Sharding over 8 NeuronCores: core c handles batch b=c//2 and head-group
hg=c%2 (8 of 16 heads). Each core computes its heads' QKV projections,
causal attention, and a partial output projection (row-sharded Wo); the
2-way reduction per batch (the "all-reduce after w_o") happens on host at
gather time, along with the bo bias (bo/2 added on each device).

Device dataflow (all bf16 matmuls, fp32 PSUM accumulate; activations kept
transposed, feature-on-partition). The schedule is software-pipelined so
the PE never idles waiting for softmax:

- Attention for q-block sb runs as a stream of (head-pair, k-chunk)
  "chunks": two K=64 score matmuls on disjoint PE row quadrants (they
  run concurrently), an additive causal-mask matmul into the diag score
  PSUM (lhsT = strict-upper * -1e9, rhs = identity), one merged exp on
  ScalarE for both heads, and two PV matmuls that lag the scores by 2
  chunks so they never wait on the exp.
- The QKV projection matmuls for block sb+1 and the Wo projection for
  block sb-1 are interleaved 1-6 per chunk into the attention stream
  ("fills"), so the exp-paced gaps are filled with useful PE work and
  the PE stays at full DVFS p-state.
- The softmax denominator rides as a ones-column in v (PSUM row 64);
  normalization is reciprocal + partition-broadcast + multiply, off the
  critical path. The final block's projection is split into hp01/hp23
  halves so most of it overlaps the last normalize chain.
"""

import math
from collections import deque

import numpy as np
import ml_dtypes
from contextlib import ExitStack

import concourse.bass as bass
import concourse.tile as tile
from concourse import bacc, mybir
from concourse.bass_utils import run_bass_kernel_spmd
from concourse.alu_op_type import AluOpType

F32 = mybir.dt.float32
BF16 = mybir.dt.bfloat16

S = 2048          # sequence length
D = 1024          # model dim
NH = 8            # heads per core
DKH = 64          # head dim
NHP = 4           # head pairs per core
SB = 512          # seq block (q block)
NSB = S // SB     # 4
KC = 128          # k chunk
NDIN = D // 128   # 8 input-dim chunks
VW = DKH + 1      # 65: v columns per head incl. ones column
NEG = -1.0e9


def build_program():
    nc = bacc.Bacc("TRN2", target_bir_lowering=False, debug=False, num_devices=8)

    xq = nc.dram_tensor("xq_t", [D, S], BF16, kind="ExternalInput").ap()
    xk = nc.dram_tensor("xk_t", [D, S], BF16, kind="ExternalInput").ap()
    xv = nc.dram_tensor("xv_t", [D, S], BF16, kind="ExternalInput").ap()
    wq = nc.dram_tensor("wq_t", [D, 512], BF16, kind="ExternalInput").ap()
    wkv = nc.dram_tensor("wkv_t", [D, 1024], BF16, kind="ExternalInput").ap()
    wo = nc.dram_tensor("wo_t", [512, D], BF16, kind="ExternalInput").ap()
    bqs = nc.dram_tensor("bq_s", [512, 1], F32, kind="ExternalInput").ap()
    bks = nc.dram_tensor("bk_s", [512, 1], F32, kind="ExternalInput").ap()
    bvr = nc.dram_tensor("bv_r", [1, 512], F32, kind="ExternalInput").ap()
    msk = nc.dram_tensor("msk", [128, 256], BF16, kind="ExternalInput").ap()
    y = nc.dram_tensor("y", [S, D], F32, kind="ExternalOutput").ap()

    with tile.TileContext(nc) as tc, ExitStack() as ctx:
        const = ctx.enter_context(tc.tile_pool(name="const", bufs=1))
        wpool = ctx.enter_context(tc.tile_pool(name="w", bufs=1))
        acts = ctx.enter_context(tc.tile_pool(name="acts", bufs=1))
        qpool = ctx.enter_context(tc.tile_pool(name="qpool", bufs=2))
        xs = ctx.enter_context(tc.tile_pool(name="xs", bufs=24))
        probs = ctx.enter_context(tc.tile_pool(name="probs", bufs=6))
        atp = ctx.enter_context(tc.tile_pool(name="at", bufs=1))
        atop = ctx.enter_context(tc.tile_pool(name="ato", bufs=2))
        nrm = ctx.enter_context(tc.tile_pool(name="nrm", bufs=4))
        ysb = ctx.enter_context(tc.tile_pool(name="ysb", bufs=4))
        y0p = ctx.enter_context(tc.tile_pool(name="y0p", bufs=1))
        mmps = ctx.enter_context(tc.tile_pool(name="mmps", bufs=2, space="PSUM"))
        scps = ctx.enter_context(tc.tile_pool(name="scps", bufs=2, space="PSUM"))
        pvps = ctx.enter_context(tc.tile_pool(name="pvps", bufs=2, space="PSUM"))

        # trigger the gpsimd custom-op library load immediately
        dum1 = const.tile([1, 16], F32, name="dum1", tag="dum1")
        nc.vector.memset(dum1[:], 0.0)
        dum2 = const.tile([8, 16], F32, name="dum2", tag="dum2")
        nc.gpsimd.partition_broadcast(dum2[:], dum1[:])
        # trigger the Exp act-table load before the first real softmax
        dume = const.tile([1, 16], F32, name="dume", tag="dume")
        nc.scalar.activation(dume[:], dum1[:], mybir.ActivationFunctionType.Exp,
                             scale=1.0)

        # ---- weights / consts, DMA'd in need-order (tiny consts first so
        # their completion sems don't chain behind the big transfers) ----
        bv_row = const.tile([1, 512], F32, name="bvr", tag="bvr")
        nc.scalar.dma_start(bv_row[:], bvr[:])
        bv_b = const.tile([128, 512], F32, name="bvb", tag="bvb")
        nc.gpsimd.partition_broadcast(bv_b[:], bv_row[:])

        # causal-mask consts: cols 0:128 = strict-upper * -1e9 (lhsT),
        # cols 128:256 = identity (rhs)
        mskt = const.tile([128, 256], BF16, name="mskt", tag="mskt")
        nc.scalar.dma_start(mskt[:], msk[:])
        triu_neg = mskt[:, 0:128]
        ident = mskt[:, 128:256]

        bq_t, bk_t = [], []
        for m_i in range(4):
            t = const.tile([128, 1], F32, name=f"bq{m_i}", tag=f"bq{m_i}")
            nc.scalar.dma_start(t[:], bqs[m_i * 128:(m_i + 1) * 128, :])
            bq_t.append(t)
            t = const.tile([128, 1], F32, name=f"bk{m_i}", tag=f"bk{m_i}")
            nc.scalar.dma_start(t[:], bks[m_i * 128:(m_i + 1) * 128, :])
            bk_t.append(t)

        wkvc = []
        for c in range(NDIN):
            t = wpool.tile([128, 1024], BF16, name=f"wkv{c}", tag=f"wkv{c}")
            nc.scalar.dma_start(t[:], wkv[c * 128:(c + 1) * 128, :])
            wkvc.append(t)

        wqc = []
        for c in range(NDIN):
            t = wpool.tile([128, 512], BF16, name=f"wq{c}", tag=f"wq{c}")
            nc.scalar.dma_start(t[:], wq[c * 128:(c + 1) * 128, :])
            wqc.append(t)

        woh2 = []
        for hp in range(NHP):
            t = wpool.tile([128, D], BF16, name=f"wo{hp}", tag=f"wo{hp}")
            nc.scalar.dma_start(t[:], wo[hp * 128:(hp + 1) * 128, :])
            woh2.append(t)

        # ---- resident k/v activations ----
        kT = {}
        for hp in range(NHP):
            for sb in range(NSB):
                kT[(hp, sb)] = acts.tile([128, SB], BF16, name=f"kT{hp}_{sb}",
                                         tag=f"kT{hp}_{sb}")
        vt = [acts.tile([128, NH * VW], BF16, name=f"v{t_i}", tag=f"v{t_i}")
              for t_i in range(S // KC)]

        qT_of = {}    # sb -> [qt tile per m_i]
        at_of = {}    # sb -> [at tile per hp]
        y0_tiles = {}

        # ---------- fill generators ----------
        class QKVGen:
            """Yields once per PE matmul of block sb's q/k/v projections."""

            def __init__(self, sb):
                self.sb = sb
                self.ssl = slice(sb * SB, (sb + 1) * SB)
                self.xk_tiles = None

            def issue_xk(self):
                self.xk_tiles = []
                for c in range(NDIN):
                    t = xs.tile([128, SB], BF16, name="x", tag="x")
                    nc.sync.dma_start(t[:], xk[c * 128:(c + 1) * 128, self.ssl])
                    self.xk_tiles.append(t)

            def issue_xv(self):
                self.xv_tiles = []
                for c in range(NDIN):
                    t = xs.tile([128, SB], BF16, name="x", tag="x")
                    nc.sync.dma_start(t[:], xv[c * 128:(c + 1) * 128, self.ssl])
                    self.xv_tiles.append(t)

            def gen(self):
                sb = self.sb
                if self.xk_tiles is None:
                    self.issue_xk()
                for m_i in range(4):
                    ps = mmps.tile([128, SB], F32, name="mm", tag="mm")
                    for c in range(NDIN):
                        nc.tensor.matmul(
                            ps[:],
                            wkvc[c][:, m_i * 128:(m_i + 1) * 128],
                            self.xk_tiles[c][:],
                            start=(c == 0), stop=(c == NDIN - 1),
                        )
                        yield
                    nc.vector.tensor_scalar_add(kT[(m_i, sb)][:], ps[:],
                                                bk_t[m_i][:])
                if not hasattr(self, "xv_tiles"):
                    self.issue_xv()
                xv_tiles = self.xv_tiles
                for m_i in range(4):
                    ps2 = mmps.tile([128, SB], F32, name="mm", tag="mm")
                    for c in range(NDIN):
                        nc.tensor.matmul(
                            ps2[:],
                            xv_tiles[c][:, m_i * 128:(m_i + 1) * 128],
                            wkvc[c][:, 512:1024],
                            start=(c == 0), stop=(c == NDIN - 1),
                        )
                        yield
                    t = vt[sb * 4 + m_i]
                    t3 = t[:].rearrange("p (h c) -> p h c", h=NH)
                    nc.vector.tensor_tensor(
                        t3[:, :, 0:DKH],
                        ps2[:].rearrange("p (h c) -> p h c", h=NH),
                        bv_b[:].rearrange("p (h c) -> p h c", h=NH),
                        AluOpType.add,
                    )
                    nc.vector.memset(t3[:, :, DKH:VW], 1.0)
                xq_tiles = []
                for c in range(NDIN):
                    t = xs.tile([128, SB], BF16, name="x", tag="x")
                    nc.sync.dma_start(t[:], xq[c * 128:(c + 1) * 128, self.ssl])
                    xq_tiles.append(t)
                qts = [None] * 4
                for m_i in range(4):
                    ps = mmps.tile([128, SB], F32, name="mm", tag="mm")
                    for c in range(NDIN):
                        nc.tensor.matmul(
                            ps[:],
                            wqc[c][:, m_i * 128:(m_i + 1) * 128],
                            xq_tiles[c][:],
                            start=(c == 0), stop=(c == NDIN - 1),
                        )
                        yield
                    qt = qpool.tile([128, SB], BF16, name=f"qT{m_i}",
                                    tag=f"qT{m_i}")
                    nc.vector.tensor_scalar_add(qt[:], ps[:], bq_t[m_i][:])
                    qts[m_i] = qt
                qT_of[sb] = qts

        def proj_gen(sb):
            """Yields once per PE matmul of block sb's output projection."""
            ats = at_of[sb]
            for m_i in range(4):
                for n_i in range(2):
                    ps = mmps.tile([128, SB], F32, name="mm", tag="mm")
                    for hp in range(NHP):
                        nc.tensor.matmul(
                            ps[:],
                            ats[hp][:, m_i * 128:(m_i + 1) * 128],
                            woh2[hp][:, n_i * SB:(n_i + 1) * SB],
                            start=(hp == 0), stop=(hp == NHP - 1),
                        )
                        yield
                    yt = ysb.tile([128, SB], F32, name="yt", tag="y")
                    nc.vector.tensor_copy(yt[:], ps[:])
                    eng = nc.sync if n_i == 0 else nc.gpsimd
                    eng.dma_start(
                        y[sb * SB + m_i * 128: sb * SB + (m_i + 1) * 128,
                          n_i * SB:(n_i + 1) * SB],
                        yt[:],
                    )

        def proj3_part1():
            """First half (hp 0,1) of block 3's projection; 16 yields."""
            ats = at_of[3]
            for m_i in range(4):
                for n_i in range(2):
                    ps = mmps.tile([128, SB], F32, name="mm", tag="mm")
                    for hp in (0, 1):
                        nc.tensor.matmul(
                            ps[:],
                            ats[hp][:, m_i * 128:(m_i + 1) * 128],
                            woh2[hp][:, n_i * SB:(n_i + 1) * SB],
                            start=(hp == 0), stop=(hp == 1),
                        )
                        yield
                    y0 = y0p.tile([128, SB], F32, name="y0",
                                  tag=f"y0_{m_i}_{n_i}")
                    nc.vector.tensor_copy(y0[:], ps[:])
                    y0_tiles[(m_i, n_i)] = y0

        def proj3_part2():
            ats = at_of[3]
            for m_i in range(4):
                for n_i in range(2):
                    ps = mmps.tile([128, SB], F32, name="mm", tag="mm")
                    for hp in (2, 3):
                        nc.tensor.matmul(
                            ps[:],
                            ats[hp][:, m_i * 128:(m_i + 1) * 128],
                            woh2[hp][:, n_i * SB:(n_i + 1) * SB],
                            start=(hp == 2), stop=(hp == 3),
                        )
                    yt = ysb.tile([128, SB], F32, name="yt", tag="y")
                    nc.vector.tensor_tensor(yt[:], ps[:],
                                            y0_tiles[(m_i, n_i)][:],
                                            AluOpType.add)
                    eng = nc.sync if n_i == 0 else nc.gpsimd
                    eng.dma_start(
                        y[3 * SB + m_i * 128: 3 * SB + (m_i + 1) * 128,
                          n_i * SB:(n_i + 1) * SB],
                        yt[:],
                    )

        class Pacer:
            def __init__(self):
                self.gens = deque()
                self.pending = 0
                self.slots_left = 0

            def add(self, gen, n):
                self.gens.append(gen)
                self.pending += n

            def draw(self, k):
                while k > 0 and self.gens:
                    try:
                        next(self.gens[0])
                        self.pending -= 1
                        k -= 1
                    except StopIteration:
                        self.gens.popleft()

            def slot(self):
                if self.slots_left > 0:
                    k = int(self.pending / self.slots_left + 0.5)
                    self.slots_left -= 1
                    self.draw(k)

            def drain(self):
                self.draw(self.pending + 1)
                self.slots_left = 0

        # ---------- attention emitters ----------
        def emit_scores(sb, hp, kc):
            kts = kT[(hp, kc // 4)]
            koff = (kc % 4) * 128
            r_i = kc - 4 * sb
            qoff = 128 * r_i if r_i > 0 else 0
            diag = r_i >= 0
            qts = qT_of[sb]
            psAB = scps.tile([128, 2 * SB], F32, name="sAB", tag="sc")
            nc.tensor.matmul(
                psAB[:, qoff:SB],
                kts[0:64, koff:koff + 128],
                qts[hp][0:64, qoff:SB],
                start=True, stop=not diag,
            )
            nc.tensor.matmul(
                psAB[:, SB + qoff:2 * SB],
                kts[64:128, koff:koff + 128],
                qts[hp][64:128, qoff:SB],
                start=True, stop=not diag,
            )
            if diag:
                # additive causal mask: psum[k, qoff+j] += -1e9 * [j < k]
                nc.tensor.matmul(
                    psAB[:, qoff:qoff + 128], triu_neg, ident,
                    start=False, stop=True, skip_group_check=True,
                )
                nc.tensor.matmul(
                    psAB[:, SB + qoff:SB + qoff + 128], triu_neg, ident,
                    start=False, stop=True, skip_group_check=True,
                )
            pAB = probs.tile([128, 2 * SB], BF16, name="pAB", tag="probs")
            ps3 = psAB[:].rearrange("p (h q) -> p h q", h=2)[:, :, qoff:SB]
            pr3 = pAB[:].rearrange("p (h q) -> p h q", h=2)[:, :, qoff:SB]
            nc.scalar.activation(pr3, ps3, mybir.ActivationFunctionType.Exp,
                                 scale=0.125)
            return (kc, pAB, qoff)

        def emit_pv(hp, item, pvA, pvB, nck):
            kc, pAB, qoff = item
            vtile = vt[kc]
            hA, hB = 2 * hp, 2 * hp + 1
            nc.tensor.matmul(
                pvA[0:VW, qoff:SB], vtile[:, hA * VW:(hA + 1) * VW],
                pAB[:, qoff:SB],
                start=(kc == 0), stop=(kc == nck - 1),
            )
            nc.tensor.matmul(
                pvB[0:VW, qoff:SB], vtile[:, hB * VW:(hB + 1) * VW],
                pAB[:, SB + qoff:2 * SB],
                start=(kc == 0), stop=(kc == nck - 1),
            )

        def emit_normalize(sb, hp, pvA, pvB):
            pvcA = nrm.tile([VW, SB], F32, name="pvcA", tag="pvc")
            nc.vector.tensor_copy(pvcA[:], pvA[0:VW, :])
            pvcB = nrm.tile([VW, SB], F32, name="pvcB", tag="pvc")
            nc.vector.tensor_copy(pvcB[:], pvB[0:VW, :])
            at = atp.tile([128, SB], BF16, name="at", tag=f"at{sb}_{hp}")
            for h_sub, pvc in ((0, pvcA), (1, pvcB)):
                rc0 = nrm.tile([1, SB], F32, name="rc0", tag="rc0")
                nc.sync.dma_start(rc0[:], pvc[64:65, :])
                rc1 = nrm.tile([1, SB], F32, name="rc1", tag="rc1")
                nc.vector.reciprocal_approx_fast(rc1[:], rc0[:])
                rb = nrm.tile([64, SB], F32, name="rb", tag="rb")
                nc.gpsimd.partition_broadcast(rb[:], rc1[:])
                if h_sub == 0:
                    nc.vector.tensor_tensor(at[0:64, :], pvc[0:64, :], rb[:],
                                            AluOpType.mult)
                else:
                    ato = atop.tile([64, SB], BF16, name="ato", tag="ato")
                    nc.vector.tensor_tensor(ato[:], pvc[0:64, :], rb[:],
                                            AluOpType.mult)
                    nc.sync.dma_start(at[64:128, :], ato[:])
            return at

        # ---------- schedule ----------
        gens = {sb: QKVGen(sb) for sb in range(NSB)}
        pacer = Pacer()

        # prologue: QKV(s0) dense; prefetch xk(s1) so sb0's fills don't stall
        gens[0].issue_xk()
        gens[0].issue_xv()
        gens[1].issue_xk()
        pacer.add(gens[0].gen(), 96)
        pacer.drain()

        for sb in range(NSB):
            if sb < NSB - 1:
                pacer.add(gens[sb + 1].gen(), 96)
            else:
                for psb in range(3):
                    pacer.add(proj_gen(psb), 32)
            nck = 4 * sb + 4
            pacer.slots_left = NHP * nck
            ats = [None] * NHP
            for hp in range(NHP):
                pvA = pvps.tile([128, SB], F32, name="pvA", tag="pv")
                pvB = pvps.tile([128, SB], F32, name="pvB", tag="pv")
                pend = deque()
                for kc in range(nck):
                    pend.append(emit_scores(sb, hp, kc))
                    if len(pend) == 3:
                        emit_pv(hp, pend.popleft(), pvA, pvB, nck)
                    pacer.slot()
                while pend:
                    emit_pv(hp, pend.popleft(), pvA, pvB, nck)
                    pacer.draw(1)
                if sb == NSB - 1 and hp == NHP - 1:
                    pacer.drain()
                ats[hp] = emit_normalize(sb, hp, pvA, pvB)
                if sb == NSB - 1 and hp == 1:
                    at_of[3] = ats  # list mutated in place as hp 2,3 finish
                    pacer.add(proj3_part1(), 16)
            at_of[sb] = ats
            pacer.drain()

        proj3_part2()

    nc.compile()
    return nc


_NC = None
_LAST_IN_MAPS = None


def _get_nc():
    global _NC
    if _NC is None:
        _NC = build_program()
    return _NC


def _build_msk():
    m = np.concatenate(
        [np.triu(np.full((128, 128), NEG, np.float32), 1),
         np.eye(128, dtype=np.float32)],
        axis=1,
    )
    return np.ascontiguousarray(m.astype(ml_dtypes.bfloat16))


def kernel(query, key, value, mask, Wq, bq, Wk, bk, Wv, bv, Wo, bo):
    query = np.asarray(query, np.float32)
    key = np.asarray(key, np.float32)
    value = np.asarray(value, np.float32)
    Wq = np.asarray(Wq, np.float32)
    Wk = np.asarray(Wk, np.float32)
    Wv = np.asarray(Wv, np.float32)
    Wo = np.asarray(Wo, np.float32)
    bq = np.asarray(bq, np.float32)
    bk = np.asarray(bk, np.float32)
    bv = np.asarray(bv, np.float32)
    bo = np.asarray(bo, np.float32)

    nc = _get_nc()

    B = query.shape[0]
    bf = ml_dtypes.bfloat16
    xq_t = [np.ascontiguousarray(query[b].T.astype(bf)) for b in range(B)]
    xk_t = [np.ascontiguousarray(key[b].T.astype(bf)) for b in range(B)]
    xv_t = [np.ascontiguousarray(value[b].T.astype(bf)) for b in range(B)]
    msk_np = _build_msk()

    in_maps = []
    for c in range(8):
        b, hg = c // 2, c % 2
        sl = slice(hg * 512, (hg + 1) * 512)
        in_maps.append({
            "xq_t": xq_t[b],
            "xk_t": xk_t[b],
            "xv_t": xv_t[b],
            "wq_t": np.ascontiguousarray(Wq[sl, :].T.astype(bf)),
            "wkv_t": np.ascontiguousarray(
                np.concatenate([Wk[sl, :].T, Wv[sl, :].T], axis=1).astype(bf)),
            "wo_t": np.ascontiguousarray(Wo[:, sl].T.astype(bf)),
            "bq_s": np.ascontiguousarray(bq[sl, None]),
            "bk_s": np.ascontiguousarray(bk[sl, None]),
            "bv_r": np.ascontiguousarray(bv[None, sl]),
            "msk": msk_np,
        })

    global _LAST_IN_MAPS
    _LAST_IN_MAPS = in_maps
    res = run_bass_kernel_spmd(nc, in_maps, core_ids=list(range(8)))
    out = np.empty((B, S, D), np.float32)
    for b in range(B):
        out[b] = res.results[2 * b]["y"] + res.results[2 * b + 1]["y"]
    out += bo[None, None, :]
    return out


# revision 10
# speedup vs baseline: 1.0850x; 1.0079x over previous
"""Trainium2 Bass kernel for a multi-head attention block (B=4, S=2048, D=1024, H=16).

Sharding over 8 NeuronCores: core c handles batch b=c//2 and head-group
hg=c%2 (8 of 16 heads). Each core computes its heads' QKV projections,
causal attention, and a partial output projection (row-sharded Wo); the
2-way reduction per batch (the "all-reduce after w_o") happens on host at
gather time, along with the bo bias (bo/2 added on each device).

Device dataflow (all bf16 matmuls, fp32 PSUM accumulate; activations kept
transposed, feature-on-partition). The schedule is software-pipelined so
the PE never idles waiting for softmax:

- Attention for q-block sb runs as a stream of (head-pair, k-chunk)
  "chunks": two K=64 score matmuls on disjoint PE row quadrants (they
  run concurrently), an additive causal-mask matmul into the diag score
  PSUM (lhsT = strict-upper * -1e9, rhs = identity), one merged exp on
  ScalarE for both heads, and two PV matmuls that lag the scores by 2
  chunks so they never wait on the exp.
- The QKV projection matmuls for block sb+1 and the Wo projection for
  block sb-1 are interleaved 1-6 per chunk into the attention stream
  ("fills"), so the exp-paced gaps are filled with useful PE work and
  the PE stays at full DVFS p-state.
- The softmax denominator rides as a ones-column in v (PSUM row 64);
  normalization is reciprocal + partition-broadcast + multiply, off the
  critical path. The final block's projection is split into hp01/hp23
  halves so most of it overlaps the last normalize chain.
"""

import math
from collections import deque

import numpy as np
import ml_dtypes
from contextlib import ExitStack

import concourse.bass as bass
import concourse.tile as tile
from concourse import bacc, mybir
from concourse.bass_utils import run_bass_kernel_spmd
from concourse.alu_op_type import AluOpType

F32 = mybir.dt.float32
BF16 = mybir.dt.bfloat16

S = 2048          # sequence length
D = 1024          # model dim
NH = 8            # heads per core
DKH = 64          # head dim
NHP = 4           # head pairs per core
SB = 512          # seq block (q block)
NSB = S // SB     # 4
KC = 128          # k chunk
NDIN = D // 128   # 8 input-dim chunks
VW = DKH + 1      # 65: v columns per head incl. ones column
NEG = -1.0e9


def build_program():
    nc = bacc.Bacc("TRN2", target_bir_lowering=False, debug=False, num_devices=8)

    xq = nc.dram_tensor("xq_t", [D, S], BF16, kind="ExternalInput").ap()
    xk = nc.dram_tensor("xk_t", [D, S], BF16, kind="ExternalInput").ap()
    xv = nc.dram_tensor("xv_t", [D, S], BF16, kind="ExternalInput").ap()
    wq = nc.dram_tensor("wq_t", [D, 512], BF16, kind="ExternalInput").ap()
    wkv = nc.dram_tensor("wkv_t", [D, 1024], BF16, kind="ExternalInput").ap()
    wo = nc.dram_tensor("wo_t", [512, D], BF16, kind="ExternalInput").ap()
    bqs = nc.dram_tensor("bq_s", [512, 1], F32, kind="ExternalInput").ap()
    bks = nc.dram_tensor("bk_s", [512, 1], F32, kind="ExternalInput").ap()
    bvr = nc.dram_tensor("bv_r", [1, 512], F32, kind="ExternalInput").ap()
    msk = nc.dram_tensor("msk", [128, 256], BF16, kind="ExternalInput").ap()
    y = nc.dram_tensor("y", [S, D], F32, kind="ExternalOutput").ap()

    with tile.TileContext(nc) as tc, ExitStack() as ctx:
        const = ctx.enter_context(tc.tile_pool(name="const", bufs=1))
        wpool = ctx.enter_context(tc.tile_pool(name="w", bufs=1))
        acts = ctx.enter_context(tc.tile_pool(name="acts", bufs=1))
        qpool = ctx.enter_context(tc.tile_pool(name="qpool", bufs=2))
        xs = ctx.enter_context(tc.tile_pool(name="xs", bufs=16))
        probs = ctx.enter_context(tc.tile_pool(name="probs", bufs=6))
        atp = ctx.enter_context(tc.tile_pool(name="at", bufs=1))
        atop = ctx.enter_context(tc.tile_pool(name="ato", bufs=2))
        nrm = ctx.enter_context(tc.tile_pool(name="nrm", bufs=4))
        ysb = ctx.enter_context(tc.tile_pool(name="ysb", bufs=4))
        y0p = ctx.enter_context(tc.tile_pool(name="y0p", bufs=1))
        mmps = ctx.enter_context(tc.tile_pool(name="mmps", bufs=2, space="PSUM"))
        scps = ctx.enter_context(tc.tile_pool(name="scps", bufs=2, space="PSUM"))
        pvps = ctx.enter_context(tc.tile_pool(name="pvps", bufs=2, space="PSUM"))

        # trigger the gpsimd custom-op library load immediately
        dum1 = const.tile([1, 16], F32, name="dum1", tag="dum1")
        nc.vector.memset(dum1[:], 0.0)
        dum2 = const.tile([8, 16], F32, name="dum2", tag="dum2")
        nc.gpsimd.partition_broadcast(dum2[:], dum1[:])

        # ---- weights / consts: issue order == ring arrival order == need
        # order. wkv + xk(s0) gate the first matmul, so they go first; the
        # Exp act-table load (1.3us of scalar time) goes after all issues.
        wkvc = []
        for c in range(NDIN):
            t = wpool.tile([128, 1024], BF16, name=f"wkv{c}", tag=f"wkv{c}")
            nc.scalar.dma_start(t[:], wkv[c * 128:(c + 1) * 128, :])
            wkvc.append(t)

        bv_row = const.tile([1, 512], F32, name="bvr", tag="bvr")
        nc.scalar.dma_start(bv_row[:], bvr[:])
        bv_b = const.tile([128, 512], F32, name="bvb", tag="bvb")
        nc.gpsimd.partition_broadcast(bv_b[:], bv_row[:])

        # causal-mask consts: cols 0:128 = strict-upper * -1e9 (lhsT),
        # cols 128:256 = identity (rhs)
        mskt = const.tile([128, 256], BF16, name="mskt", tag="mskt")
        nc.scalar.dma_start(mskt[:], msk[:])
        triu_neg = mskt[:, 0:128]
        ident = mskt[:, 128:256]

        bq_t, bk_t = [], []
        for m_i in range(4):
            t = const.tile([128, 1], F32, name=f"bq{m_i}", tag=f"bq{m_i}")
            nc.scalar.dma_start(t[:], bqs[m_i * 128:(m_i + 1) * 128, :])
            bq_t.append(t)
            t = const.tile([128, 1], F32, name=f"bk{m_i}", tag=f"bk{m_i}")
            nc.scalar.dma_start(t[:], bks[m_i * 128:(m_i + 1) * 128, :])
            bk_t.append(t)

        wqc = []
        woh2 = []

        # ---- resident k/v activations ----
        kT = {}
        for hp in range(NHP):
            for sb in range(NSB):
                kT[(hp, sb)] = acts.tile([128, SB], BF16, name=f"kT{hp}_{sb}",
                                         tag=f"kT{hp}_{sb}")
        vt = [acts.tile([128, NH * VW], BF16, name=f"v{t_i}", tag=f"v{t_i}")
              for t_i in range(S // KC)]

        qT_of = {}    # sb -> [qt tile per m_i]
        at_of = {}    # sb -> [at tile per hp]
        y0_tiles = {}

        # ---------- fill generators ----------
        class QKVGen:
            """Yields once per PE matmul of block sb's q/k/v projections."""

            def __init__(self, sb):
                self.sb = sb
                self.ssl = slice(sb * SB, (sb + 1) * SB)
                self.xk_tiles = None

            def issue_xk(self):
                self.xk_tiles = []
                for c in range(NDIN):
                    t = xs.tile([128, SB], BF16, name="x", tag="x")
                    nc.sync.dma_start(t[:], xk[c * 128:(c + 1) * 128, self.ssl])
                    self.xk_tiles.append(t)

            def issue_xv(self, eng=None):
                eng = eng or nc.sync
                self.xv_tiles = []
                for c in range(NDIN):
                    t = xs.tile([128, SB], BF16, name="x", tag="x")
                    eng.dma_start(t[:], xv[c * 128:(c + 1) * 128, self.ssl])
                    self.xv_tiles.append(t)

            def gen(self):
                sb = self.sb
                if self.xk_tiles is None:
                    self.issue_xk()
                for m_i in range(4):
                    ps = mmps.tile([128, SB], F32, name="mm", tag="mm")
                    for c in range(NDIN):
                        nc.tensor.matmul(
                            ps[:],
                            wkvc[c][:, m_i * 128:(m_i + 1) * 128],
                            self.xk_tiles[c][:],
                            start=(c == 0), stop=(c == NDIN - 1),
                        )
                        yield
                    nc.vector.tensor_scalar_add(kT[(m_i, sb)][:], ps[:],
                                                bk_t[m_i][:])
                if not hasattr(self, "xv_tiles"):
                    self.issue_xv()
                xv_tiles = self.xv_tiles
                for m_i in range(4):
                    ps2 = mmps.tile([128, SB], F32, name="mm", tag="mm")
                    for c in range(NDIN):
                        nc.tensor.matmul(
                            ps2[:],
                            xv_tiles[c][:, m_i * 128:(m_i + 1) * 128],
                            wkvc[c][:, 512:1024],
                            start=(c == 0), stop=(c == NDIN - 1),
                        )
                        yield
                    t = vt[sb * 4 + m_i]
                    t3 = t[:].rearrange("p (h c) -> p h c", h=NH)
                    nc.vector.tensor_tensor(
                        t3[:, :, 0:DKH],
                        ps2[:].rearrange("p (h c) -> p h c", h=NH),
                        bv_b[:].rearrange("p (h c) -> p h c", h=NH),
                        AluOpType.add,
                    )
                    nc.vector.memset(t3[:, :, DKH:VW], 1.0)
                xq_tiles = []
                for c in range(NDIN):
                    t = xs.tile([128, SB], BF16, name="x", tag="x")
                    nc.sync.dma_start(t[:], xq[c * 128:(c + 1) * 128, self.ssl])
                    xq_tiles.append(t)
                qts = [None] * 4
                for m_i in range(4):
                    ps = mmps.tile([128, SB], F32, name="mm", tag="mm")
                    for c in range(NDIN):
                        nc.tensor.matmul(
                            ps[:],
                            wqc[c][:, m_i * 128:(m_i + 1) * 128],
                            xq_tiles[c][:],
                            start=(c == 0), stop=(c == NDIN - 1),
                        )
                        yield
                    qt = qpool.tile([128, SB], BF16, name=f"qT{m_i}",
                                    tag=f"qT{m_i}")
                    nc.vector.tensor_scalar_add(qt[:], ps[:], bq_t[m_i][:])
                    qts[m_i] = qt
                qT_of[sb] = qts

        def proj_gen(sb):
            """Yields once per PE matmul of block sb's output projection."""
            ats = at_of[sb]
            for m_i in range(4):
                for n_i in range(2):
                    ps = mmps.tile([128, SB], F32, name="mm", tag="mm")
                    for hp in range(NHP):
                        nc.tensor.matmul(
                            ps[:],
                            ats[hp][:, m_i * 128:(m_i + 1) * 128],
                            woh2[hp][:, n_i * SB:(n_i + 1) * SB],
                            start=(hp == 0), stop=(hp == NHP - 1),
                        )
                        yield
                    yt = ysb.tile([128, SB], F32, name="yt", tag="y")
                    nc.vector.tensor_copy(yt[:], ps[:])
                    eng = nc.sync if n_i == 0 else nc.gpsimd
                    eng.dma_start(
                        y[sb * SB + m_i * 128: sb * SB + (m_i + 1) * 128,
                          n_i * SB:(n_i + 1) * SB],
                        yt[:],
                    )

        def proj3_part1():
            """First half (hp 0,1) of block 3's projection; 16 yields."""
            ats = at_of[3]
            for m_i in range(4):
                for n_i in range(2):
                    ps = mmps.tile([128, SB], F32, name="mm", tag="mm")
                    for hp in (0, 1):
                        nc.tensor.matmul(
                            ps[:],
                            ats[hp][:, m_i * 128:(m_i + 1) * 128],
                            woh2[hp][:, n_i * SB:(n_i + 1) * SB],
                            start=(hp == 0), stop=(hp == 1),
                        )
                        yield
                    y0 = y0p.tile([128, SB], F32, name="y0",
                                  tag=f"y0_{m_i}_{n_i}")
                    nc.vector.tensor_copy(y0[:], ps[:])
                    y0_tiles[(m_i, n_i)] = y0

        def proj3_part2():
            ats = at_of[3]
            for m_i in range(4):
                for n_i in range(2):
                    ps = mmps.tile([128, SB], F32, name="mm", tag="mm")
                    for hp in (2, 3):
                        nc.tensor.matmul(
                            ps[:],
                            ats[hp][:, m_i * 128:(m_i + 1) * 128],
                            woh2[hp][:, n_i * SB:(n_i + 1) * SB],
                            start=(hp == 2), stop=(hp == 3),
                        )
                    yt = ysb.tile([128, SB], F32, name="yt", tag="y")
                    nc.vector.tensor_tensor(yt[:], ps[:],
                                            y0_tiles[(m_i, n_i)][:],
                                            AluOpType.add)
                    eng = nc.sync
                    eng.dma_start(
                        y[3 * SB + m_i * 128: 3 * SB + (m_i + 1) * 128,
                          n_i * SB:(n_i + 1) * SB],
                        yt[:],
                    )

        class Pacer:
            def __init__(self):
                self.gens = deque()
                self.pending = 0
                self.slots_left = 0
                self.reserve = 0

            def add(self, gen, n):
                self.gens.append(gen)
                self.pending += n

            def draw(self, k):
                while k > 0 and self.gens:
                    try:
                        next(self.gens[0])
                        self.pending -= 1
                        k -= 1
                    except StopIteration:
                        self.gens.popleft()

            def slot(self):
                if self.slots_left > 0:
                    avail = max(0, self.pending - self.reserve)
                    k = int(avail / self.slots_left + 0.5)
                    self.slots_left -= 1
                    self.draw(k)

            def drain(self):
                self.draw(self.pending + 1)
                self.slots_left = 0

        # ---------- attention emitters ----------
        def emit_scores(sb, hp, kc):
            kts = kT[(hp, kc // 4)]
            koff = (kc % 4) * 128
            r_i = kc - 4 * sb
            qoff = 128 * r_i if r_i > 0 else 0
            diag = r_i >= 0
            qts = qT_of[sb]
            psAB = scps.tile([128, 2 * SB], F32, name="sAB", tag="sc")
            nc.tensor.matmul(
                psAB[:, qoff:SB],
                kts[0:64, koff:koff + 128],
                qts[hp][0:64, qoff:SB],
                start=True, stop=not diag,
            )
            nc.tensor.matmul(
                psAB[:, SB + qoff:2 * SB],
                kts[64:128, koff:koff + 128],
                qts[hp][64:128, qoff:SB],
                start=True, stop=not diag,
            )
            if diag:
                # additive causal mask: psum[k, qoff+j] += -1e9 * [j < k]
                nc.tensor.matmul(
                    psAB[:, qoff:qoff + 128], triu_neg, ident,
                    start=False, stop=True, skip_group_check=True,
                )
                nc.tensor.matmul(
                    psAB[:, SB + qoff:SB + qoff + 128], triu_neg, ident,
                    start=False, stop=True, skip_group_check=True,
                )
            pAB = probs.tile([128, 2 * SB], BF16, name="pAB", tag="probs")
            ps3 = psAB[:].rearrange("p (h q) -> p h q", h=2)[:, :, qoff:SB]
            pr3 = pAB[:].rearrange("p (h q) -> p h q", h=2)[:, :, qoff:SB]
            nc.scalar.activation(pr3, ps3, mybir.ActivationFunctionType.Exp,
                                 scale=0.125)
            return (kc, pAB, qoff)

        def emit_pv(hp, item, pvA, pvB, nck):
            kc, pAB, qoff = item
            vtile = vt[kc]
            hA, hB = 2 * hp, 2 * hp + 1
            nc.tensor.matmul(
                pvA[0:VW, qoff:SB], vtile[:, hA * VW:(hA + 1) * VW],
                pAB[:, qoff:SB],
                start=(kc == 0), stop=(kc == nck - 1),
            )
            nc.tensor.matmul(
                pvB[0:VW, qoff:SB], vtile[:, hB * VW:(hB + 1) * VW],
                pAB[:, SB + qoff:2 * SB],
                start=(kc == 0), stop=(kc == nck - 1),
            )

        def emit_normalize(sb, hp, pvA, pvB):
            pvcA = nrm.tile([VW, SB], F32, name="pvcA", tag="pvc")
            nc.vector.tensor_copy(pvcA[:], pvA[0:VW, :])
            pvcB = nrm.tile([VW, SB], F32, name="pvcB", tag="pvc")
            nc.vector.tensor_copy(pvcB[:], pvB[0:VW, :])
            at = atp.tile([128, SB], BF16, name="at", tag=f"at{sb}_{hp}")
            for h_sub, pvc in ((0, pvcA), (1, pvcB)):
                rc0 = nrm.tile([1, SB], F32, name="rc0", tag="rc0")
                nc.sync.dma_start(rc0[:], pvc[64:65, :])
                rc1 = nrm.tile([1, SB], F32, name="rc1", tag="rc1")
                nc.vector.reciprocal_approx_fast(rc1[:], rc0[:])
                rb = nrm.tile([64, SB], F32, name="rb", tag="rb")
                nc.gpsimd.partition_broadcast(rb[:], rc1[:])
                if h_sub == 0:
                    nc.vector.tensor_tensor(at[0:64, :], pvc[0:64, :], rb[:],
                                            AluOpType.mult)
                else:
                    ato = atop.tile([64, SB], BF16, name="ato", tag="ato")
                    nc.vector.tensor_tensor(ato[:], pvc[0:64, :], rb[:],
                                            AluOpType.mult)
                    nc.sync.dma_start(at[64:128, :], ato[:])
            return at

        # ---------- schedule ----------
        gens = {sb: QKVGen(sb) for sb in range(NSB)}
        pacer = Pacer()

        # prologue: QKV(s0) dense. Issue streams are ordered by need; the
        # xs pool (bufs=16) naturally delays gen1's xk DMAs until the kT
        # phase frees gen0's xk tiles, keeping the rings clear at startup.
        gens[0].issue_xk()
        gens[0].issue_xv(nc.scalar)

        for c in range(NDIN):
            t = wpool.tile([128, 512], BF16, name=f"wq{c}", tag=f"wq{c}")
            nc.scalar.dma_start(t[:], wq[c * 128:(c + 1) * 128, :])
            wqc.append(t)
        for hp in range(NHP):
            t = wpool.tile([128, D], BF16, name=f"wo{hp}", tag=f"wo{hp}")
            nc.scalar.dma_start(t[:], wo[hp * 128:(hp + 1) * 128, :])
            woh2.append(t)
        # trigger the Exp act-table load before the first real softmax
        dume = const.tile([1, 16], F32, name="dume", tag="dume")
        nc.scalar.activation(dume[:], dum1[:], mybir.ActivationFunctionType.Exp,
                             scale=1.0)

        gens[1].issue_xk()
        pacer.add(gens[0].gen(), 96)
        pacer.drain()

        for sb in range(NSB):
            if sb < NSB - 1:
                pacer.add(gens[sb + 1].gen(), 96)
            else:
                for psb in range(3):
                    pacer.add(proj_gen(psb), 32)
            nck = 4 * sb + 4
            pacer.slots_left = NHP * nck
            if sb == NSB - 1:
                pacer.reserve = 12
            ats = [None] * NHP
            for hp in range(NHP):
                pvA = pvps.tile([128, SB], F32, name="pvA", tag="pv")
                pvB = pvps.tile([128, SB], F32, name="pvB", tag="pv")
                pend = deque()
                for kc in range(nck):
                    pend.append(emit_scores(sb, hp, kc))
                    if len(pend) == 3:
                        emit_pv(hp, pend.popleft(), pvA, pvB, nck)
                    pacer.slot()
                while pend:
                    emit_pv(hp, pend.popleft(), pvA, pvB, nck)
                    pacer.draw(1)
                ats[hp] = emit_normalize(sb, hp, pvA, pvB)
                if sb == NSB - 1 and hp == NHP - 1:
                    pacer.reserve = 0
                    pacer.drain()
                if sb == NSB - 1 and hp == 1:
                    at_of[3] = ats  # list mutated in place as hp 2,3 finish
                    pacer.add(proj3_part1(), 16)
            at_of[sb] = ats
            pacer.drain()

        proj3_part2()

    nc.compile()
    return nc


_NC = None
_LAST_IN_MAPS = None


def _get_nc():
    global _NC
    if _NC is None:
        _NC = build_program()
    return _NC


def _build_msk():
    m = np.concatenate(
        [np.triu(np.full((128, 128), NEG, np.float32), 1),
         np.eye(128, dtype=np.float32)],
        axis=1,
    )
    return np.ascontiguousarray(m.astype(ml_dtypes.bfloat16))


def kernel(query, key, value, mask, Wq, bq, Wk, bk, Wv, bv, Wo, bo):
    query = np.asarray(query, np.float32)
    key = np.asarray(key, np.float32)
    value = np.asarray(value, np.float32)
    Wq = np.asarray(Wq, np.float32)
    Wk = np.asarray(Wk, np.float32)
    Wv = np.asarray(Wv, np.float32)
    Wo = np.asarray(Wo, np.float32)
    bq = np.asarray(bq, np.float32)
    bk = np.asarray(bk, np.float32)
    bv = np.asarray(bv, np.float32)
    bo = np.asarray(bo, np.float32)

    nc = _get_nc()

    B = query.shape[0]
    bf = ml_dtypes.bfloat16
    xq_t = [np.ascontiguousarray(query[b].T.astype(bf)) for b in range(B)]
    xk_t = [np.ascontiguousarray(key[b].T.astype(bf)) for b in range(B)]
    xv_t = [np.ascontiguousarray(value[b].T.astype(bf)) for b in range(B)]
    msk_np = _build_msk()

    in_maps = []
    for c in range(8):
        b, hg = c // 2, c % 2
        sl = slice(hg * 512, (hg + 1) * 512)
        in_maps.append({
            "xq_t": xq_t[b],
            "xk_t": xk_t[b],
            "xv_t": xv_t[b],
            "wq_t": np.ascontiguousarray(Wq[sl, :].T.astype(bf)),
            "wkv_t": np.ascontiguousarray(
                np.concatenate([Wk[sl, :].T, Wv[sl, :].T], axis=1).astype(bf)),
            "wo_t": np.ascontiguousarray(Wo[:, sl].T.astype(bf)),
            "bq_s": np.ascontiguousarray(bq[sl, None]),
            "bk_s": np.ascontiguousarray(bk[sl, None]),
            "bv_r": np.ascontiguousarray(bv[None, sl]),
            "msk": msk_np,
        })

    global _LAST_IN_MAPS
    _LAST_IN_MAPS = in_maps
    res = run_bass_kernel_spmd(nc, in_maps, core_ids=list(range(8)))
    out = np.empty((B, S, D), np.float32)
    for b in range(B):
        out[b] = res.results[2 * b]["y"] + res.results[2 * b + 1]["y"]
    out += bo[None, None, :]
    return out


# revision 11
# speedup vs baseline: 1.0898x; 1.0044x over previous
"""Trainium2 Bass kernel for a multi-head attention block (B=4, S=2048, D=1024, H=16).

Sharding over 8 NeuronCores: core c handles batch b=c//2 and head-group
hg=c%2 (8 of 16 heads). Each core computes its heads' QKV projections,
causal attention, and a partial output projection (row-sharded Wo); the
2-way reduction per batch (the "all-reduce after w_o") happens on host at
gather time, along with the bo bias (bo/2 added on each device).

Device dataflow (all bf16 matmuls, fp32 PSUM accumulate; activations kept
transposed, feature-on-partition). The schedule is software-pipelined so
the PE never idles waiting for softmax:

- Attention for q-block sb runs as a stream of (head-pair, k-chunk)
  "chunks": two K=64 score matmuls on disjoint PE row quadrants (they
  run concurrently), an additive causal-mask matmul into the diag score
  PSUM (lhsT = strict-upper * -1e9, rhs = identity), one merged exp on
  ScalarE for both heads, and two PV matmuls that lag the scores by 2
  chunks so they never wait on the exp.
- The QKV projection matmuls for block sb+1 and the Wo projection for
  block sb-1 are interleaved 1-6 per chunk into the attention stream
  ("fills"), so the exp-paced gaps are filled with useful PE work and
  the PE stays at full DVFS p-state.
- The softmax denominator rides as a ones-column in v (PSUM row 64);
  normalization is reciprocal + partition-broadcast + multiply, off the
  critical path. The final block's projection is split into hp01/hp23
  halves so most of it overlaps the last normalize chain.
"""

import math
from collections import deque

import numpy as np
import ml_dtypes
from contextlib import ExitStack

import concourse.bass as bass
import concourse.tile as tile
from concourse import bacc, mybir
from concourse.bass_utils import run_bass_kernel_spmd
from concourse.alu_op_type import AluOpType

F32 = mybir.dt.float32
BF16 = mybir.dt.bfloat16

S = 2048          # sequence length
D = 1024          # model dim
NH = 8            # heads per core
DKH = 64          # head dim
NHP = 4           # head pairs per core
SB = 512          # seq block (q block)
NSB = S // SB     # 4
KC = 128          # k chunk
NDIN = D // 128   # 8 input-dim chunks
VW = DKH + 1      # 65: v columns per head incl. ones column
NEG = -1.0e9


def build_program():
    nc = bacc.Bacc("TRN2", target_bir_lowering=False, debug=False, num_devices=8)

    xq = nc.dram_tensor("xq_t", [D, S], BF16, kind="ExternalInput").ap()
    xk = nc.dram_tensor("xk_t", [D, S], BF16, kind="ExternalInput").ap()
    xv = nc.dram_tensor("xv_t", [D, S], BF16, kind="ExternalInput").ap()
    wq = nc.dram_tensor("wq_t", [D, 512], BF16, kind="ExternalInput").ap()
    wkv = nc.dram_tensor("wkv_t", [D, 1024], BF16, kind="ExternalInput").ap()
    wo = nc.dram_tensor("wo_t", [512, D], BF16, kind="ExternalInput").ap()
    bqs = nc.dram_tensor("bq_s", [512, 1], F32, kind="ExternalInput").ap()
    bks = nc.dram_tensor("bk_s", [512, 1], F32, kind="ExternalInput").ap()
    bvr = nc.dram_tensor("bv_r", [1, 512], F32, kind="ExternalInput").ap()
    msk = nc.dram_tensor("msk", [128, 256], BF16, kind="ExternalInput").ap()
    y = nc.dram_tensor("y", [S, D], F32, kind="ExternalOutput").ap()

    with tile.TileContext(nc) as tc, ExitStack() as ctx:
        const = ctx.enter_context(tc.tile_pool(name="const", bufs=1))
        wpool = ctx.enter_context(tc.tile_pool(name="w", bufs=1))
        acts = ctx.enter_context(tc.tile_pool(name="acts", bufs=1))
        qpool = ctx.enter_context(tc.tile_pool(name="qpool", bufs=2))
        xs = ctx.enter_context(tc.tile_pool(name="xs", bufs=24))
        probs = ctx.enter_context(tc.tile_pool(name="probs", bufs=8))
        atp = ctx.enter_context(tc.tile_pool(name="at", bufs=1))
        atop = ctx.enter_context(tc.tile_pool(name="ato", bufs=2))
        nrm = ctx.enter_context(tc.tile_pool(name="nrm", bufs=4))
        ysb = ctx.enter_context(tc.tile_pool(name="ysb", bufs=4))
        y0p = ctx.enter_context(tc.tile_pool(name="y0p", bufs=1))
        mmps = ctx.enter_context(tc.tile_pool(name="mmps", bufs=2, space="PSUM"))
        scps = ctx.enter_context(tc.tile_pool(name="scps", bufs=2, space="PSUM"))
        pvps = ctx.enter_context(tc.tile_pool(name="pvps", bufs=2, space="PSUM"))

        # trigger the gpsimd custom-op library load immediately
        dum1 = const.tile([1, 16], F32, name="dum1", tag="dum1")
        nc.vector.memset(dum1[:], 0.0)
        dum2 = const.tile([8, 16], F32, name="dum2", tag="dum2")
        nc.gpsimd.partition_broadcast(dum2[:], dum1[:])

        # ---- weights / consts: issue order == ring arrival order == need
        # order. wkv + xk(s0) gate the first matmul, so they go first; the
        # Exp act-table load (1.3us of scalar time) goes after all issues.
        wkvc = []
        for c in range(NDIN):
            t = wpool.tile([128, 1024], BF16, name=f"wkv{c}", tag=f"wkv{c}")
            nc.scalar.dma_start(t[:], wkv[c * 128:(c + 1) * 128, :])
            wkvc.append(t)

        bv_row = const.tile([1, 512], F32, name="bvr", tag="bvr")
        nc.scalar.dma_start(bv_row[:], bvr[:])
        bv_b = const.tile([128, 512], F32, name="bvb", tag="bvb")
        nc.gpsimd.partition_broadcast(bv_b[:], bv_row[:])

        # causal-mask consts: cols 0:128 = strict-upper * -1e9 (lhsT),
        # cols 128:256 = identity (rhs)
        mskt = const.tile([128, 256], BF16, name="mskt", tag="mskt")
        nc.scalar.dma_start(mskt[:], msk[:])
        triu_neg = mskt[:, 0:128]
        ident = mskt[:, 128:256]

        bq_t, bk_t = [], []
        for m_i in range(4):
            t = const.tile([128, 1], F32, name=f"bq{m_i}", tag=f"bq{m_i}")
            nc.scalar.dma_start(t[:], bqs[m_i * 128:(m_i + 1) * 128, :])
            bq_t.append(t)
            t = const.tile([128, 1], F32, name=f"bk{m_i}", tag=f"bk{m_i}")
            nc.scalar.dma_start(t[:], bks[m_i * 128:(m_i + 1) * 128, :])
            bk_t.append(t)

        wqc = []
        woh2 = []

        # ---- resident k/v activations ----
        kT = {}
        for hp in range(NHP):
            for sb in range(NSB):
                kT[(hp, sb)] = acts.tile([128, SB], BF16, name=f"kT{hp}_{sb}",
                                         tag=f"kT{hp}_{sb}")
        vt = [acts.tile([128, NH * VW], BF16, name=f"v{t_i}", tag=f"v{t_i}")
              for t_i in range(S // KC)]

        qT_of = {}    # sb -> [qt tile per m_i]
        at_of = {}    # sb -> [at tile per hp]
        y0_tiles = {}

        # ---------- fill generators ----------
        class QKVGen:
            """Yields once per PE matmul of block sb's q/k/v projections."""

            def __init__(self, sb):
                self.sb = sb
                self.ssl = slice(sb * SB, (sb + 1) * SB)
                self.xk_tiles = None

            def issue_xk(self):
                self.xk_tiles = []
                for c in range(NDIN):
                    t = xs.tile([128, SB], BF16, name="x", tag="x")
                    nc.sync.dma_start(t[:], xk[c * 128:(c + 1) * 128, self.ssl])
                    self.xk_tiles.append(t)

            def issue_xv(self, eng=None):
                eng = eng or nc.sync
                self.xv_tiles = []
                for c in range(NDIN):
                    t = xs.tile([128, SB], BF16, name="x", tag="x")
                    eng.dma_start(t[:], xv[c * 128:(c + 1) * 128, self.ssl])
                    self.xv_tiles.append(t)

            def gen(self):
                sb = self.sb
                if self.xk_tiles is None:
                    self.issue_xk()
                for m_i in range(4):
                    ps = mmps.tile([128, SB], F32, name="mm", tag="mm")
                    for c in range(NDIN):
                        nc.tensor.matmul(
                            ps[:],
                            wkvc[c][:, m_i * 128:(m_i + 1) * 128],
                            self.xk_tiles[c][:],
                            start=(c == 0), stop=(c == NDIN - 1),
                        )
                        yield
                    nc.vector.tensor_scalar_add(kT[(m_i, sb)][:], ps[:],
                                                bk_t[m_i][:])
                if not hasattr(self, "xv_tiles"):
                    self.issue_xv()
                xv_tiles = self.xv_tiles
                for m_i in range(4):
                    ps2 = mmps.tile([128, SB], F32, name="mm", tag="mm")
                    for c in range(NDIN):
                        nc.tensor.matmul(
                            ps2[:],
                            xv_tiles[c][:, m_i * 128:(m_i + 1) * 128],
                            wkvc[c][:, 512:1024],
                            start=(c == 0), stop=(c == NDIN - 1),
                        )
                        yield
                    t = vt[sb * 4 + m_i]
                    t3 = t[:].rearrange("p (h c) -> p h c", h=NH)
                    nc.vector.tensor_tensor(
                        t3[:, :, 0:DKH],
                        ps2[:].rearrange("p (h c) -> p h c", h=NH),
                        bv_b[:].rearrange("p (h c) -> p h c", h=NH),
                        AluOpType.add,
                    )
                    nc.vector.memset(t3[:, :, DKH:VW], 1.0)
                xq_tiles = []
                for c in range(NDIN):
                    t = xs.tile([128, SB], BF16, name="x", tag="x")
                    nc.sync.dma_start(t[:], xq[c * 128:(c + 1) * 128, self.ssl])
                    xq_tiles.append(t)
                qts = [None] * 4
                for m_i in range(4):
                    ps = mmps.tile([128, SB], F32, name="mm", tag="mm")
                    for c in range(NDIN):
                        nc.tensor.matmul(
                            ps[:],
                            wqc[c][:, m_i * 128:(m_i + 1) * 128],
                            xq_tiles[c][:],
                            start=(c == 0), stop=(c == NDIN - 1),
                        )
                        yield
                    qt = qpool.tile([128, SB], BF16, name=f"qT{m_i}",
                                    tag=f"qT{m_i}")
                    nc.vector.tensor_scalar_add(qt[:], ps[:], bq_t[m_i][:])
                    qts[m_i] = qt
                qT_of[sb] = qts

        def proj_gen(sb):
            """Yields once per PE matmul of block sb's output projection."""
            ats = at_of[sb]
            for m_i in range(4):
                for n_i in range(2):
                    ps = mmps.tile([128, SB], F32, name="mm", tag="mm")
                    for hp in range(NHP):
                        nc.tensor.matmul(
                            ps[:],
                            ats[hp][:, m_i * 128:(m_i + 1) * 128],
                            woh2[hp][:, n_i * SB:(n_i + 1) * SB],
                            start=(hp == 0), stop=(hp == NHP - 1),
                        )
                        yield
                    yt = ysb.tile([128, SB], F32, name="yt", tag="y")
                    nc.vector.tensor_copy(yt[:], ps[:])
                    eng = nc.sync if n_i == 0 else nc.gpsimd
                    eng.dma_start(
                        y[sb * SB + m_i * 128: sb * SB + (m_i + 1) * 128,
                          n_i * SB:(n_i + 1) * SB],
                        yt[:],
                    )

        def proj3_part1():
            """First half (hp 0,1) of block 3's projection; 16 yields."""
            ats = at_of[3]
            for m_i in range(4):
                for n_i in range(2):
                    ps = mmps.tile([128, SB], F32, name="mm", tag="mm")
                    for hp in (0, 1):
                        nc.tensor.matmul(
                            ps[:],
                            ats[hp][:, m_i * 128:(m_i + 1) * 128],
                            woh2[hp][:, n_i * SB:(n_i + 1) * SB],
                            start=(hp == 0), stop=(hp == 1),
                        )
                        yield
                    y0 = y0p.tile([128, SB], BF16, name="y0",
                                  tag=f"y0_{m_i}_{n_i}")
                    nc.vector.tensor_copy(y0[:], ps[:])
                    y0_tiles[(m_i, n_i)] = y0

        def proj3_part2():
            # hp2/hp3 accumulate on top of part1's partial (re-injected with
            # an identity matmul), evac via the idle Scalar engine so the
            # tail never waits on the Vector queue.
            ats = at_of[3]
            for m_i in range(4):
                for n_i in range(2):
                    ps = mmps.tile([128, SB], F32, name="mm", tag="mm")
                    nc.tensor.matmul(ps[:], ident, y0_tiles[(m_i, n_i)][:],
                                     start=True, stop=False,
                                     skip_group_check=True)
                    for hp in (2, 3):
                        nc.tensor.matmul(
                            ps[:],
                            ats[hp][:, m_i * 128:(m_i + 1) * 128],
                            woh2[hp][:, n_i * SB:(n_i + 1) * SB],
                            start=False, stop=(hp == 3),
                            skip_group_check=True,
                        )
                    yt = ysb.tile([128, SB], F32, name="yt", tag="y")
                    nc.scalar.activation(yt[:], ps[:],
                                         mybir.ActivationFunctionType.Copy)
                    nc.sync.dma_start(
                        y[3 * SB + m_i * 128: 3 * SB + (m_i + 1) * 128,
                          n_i * SB:(n_i + 1) * SB],
                        yt[:],
                    )

        class Pacer:
            def __init__(self):
                self.gens = deque()
                self.pending = 0
                self.slots_left = 0
                self.reserve = 0

            def add(self, gen, n, front=False):
                if front:
                    self.gens.appendleft(gen)
                else:
                    self.gens.append(gen)
                self.pending += n

            def draw(self, k):
                while k > 0 and self.gens:
                    try:
                        next(self.gens[0])
                        self.pending -= 1
                        k -= 1
                    except StopIteration:
                        self.gens.popleft()

            def slot(self):
                if self.slots_left > 0:
                    avail = max(0, self.pending - self.reserve)
                    k = int(avail / self.slots_left + 0.5)
                    self.slots_left -= 1
                    self.draw(k)

            def drain(self):
                self.draw(self.pending + 1)
                self.slots_left = 0

        # ---------- attention emitters ----------
        def emit_score_mms(sb, hp, kc):
            kts = kT[(hp, kc // 4)]
            koff = (kc % 4) * 128
            r_i = kc - 4 * sb
            qoff = 128 * r_i if r_i > 0 else 0
            diag = r_i >= 0
            qts = qT_of[sb]
            psAB = scps.tile([128, 2 * SB], F32, name="sAB", tag="sc")
            nc.tensor.matmul(
                psAB[:, qoff:SB],
                kts[0:64, koff:koff + 128],
                qts[hp][0:64, qoff:SB],
                start=True, stop=not diag,
            )
            nc.tensor.matmul(
                psAB[:, SB + qoff:2 * SB],
                kts[64:128, koff:koff + 128],
                qts[hp][64:128, qoff:SB],
                start=True, stop=not diag,
            )
            return (kc, psAB, qoff, diag)

        def emit_mask_exp(item):
            kc, psAB, qoff, diag = item
            if diag:
                # additive causal mask: psum[k, qoff+j] += -1e9 * [j < k]
                nc.tensor.matmul(
                    psAB[:, qoff:qoff + 128], triu_neg, ident,
                    start=False, stop=True, skip_group_check=True,
                )
                nc.tensor.matmul(
                    psAB[:, SB + qoff:SB + qoff + 128], triu_neg, ident,
                    start=False, stop=True, skip_group_check=True,
                )
            pAB = probs.tile([128, 2 * SB], BF16, name="pAB", tag="probs")
            ps3 = psAB[:].rearrange("p (h q) -> p h q", h=2)[:, :, qoff:SB]
            pr3 = pAB[:].rearrange("p (h q) -> p h q", h=2)[:, :, qoff:SB]
            nc.scalar.activation(pr3, ps3, mybir.ActivationFunctionType.Exp,
                                 scale=0.125)
            return (kc, pAB, qoff)

        def emit_pv(hp, item, pvA, pvB, nck):
            kc, pAB, qoff = item
            vtile = vt[kc]
            hA, hB = 2 * hp, 2 * hp + 1
            nc.tensor.matmul(
                pvA[0:VW, qoff:SB], vtile[:, hA * VW:(hA + 1) * VW],
                pAB[:, qoff:SB],
                start=(kc == 0), stop=(kc == nck - 1),
            )
            nc.tensor.matmul(
                pvB[0:VW, qoff:SB], vtile[:, hB * VW:(hB + 1) * VW],
                pAB[:, SB + qoff:2 * SB],
                start=(kc == 0), stop=(kc == nck - 1),
            )

        def emit_normalize(sb, hp, pvA, pvB):
            pvcA = nrm.tile([VW, SB], F32, name="pvcA", tag="pvc")
            nc.vector.tensor_copy(pvcA[:], pvA[0:VW, :])
            pvcB = nrm.tile([VW, SB], F32, name="pvcB", tag="pvc")
            nc.vector.tensor_copy(pvcB[:], pvB[0:VW, :])
            at = atp.tile([128, SB], BF16, name="at", tag=f"at{sb}_{hp}")
            for h_sub, pvc in ((0, pvcA), (1, pvcB)):
                rc0 = nrm.tile([1, SB], F32, name="rc0", tag="rc0")
                nc.sync.dma_start(rc0[:], pvc[64:65, :])
                rc1 = nrm.tile([1, SB], F32, name="rc1", tag="rc1")
                nc.vector.reciprocal_approx_fast(rc1[:], rc0[:])
                rb = nrm.tile([64, SB], F32, name="rb", tag="rb")
                nc.gpsimd.partition_broadcast(rb[:], rc1[:])
                if h_sub == 0:
                    nc.vector.tensor_tensor(at[0:64, :], pvc[0:64, :], rb[:],
                                            AluOpType.mult)
                else:
                    ato = atop.tile([64, SB], BF16, name="ato", tag="ato")
                    nc.vector.tensor_tensor(ato[:], pvc[0:64, :], rb[:],
                                            AluOpType.mult)
                    nc.sync.dma_start(at[64:128, :], ato[:])
            return at

        # ---------- schedule ----------
        gens = {sb: QKVGen(sb) for sb in range(NSB)}
        pacer = Pacer()

        # prologue: QKV(s0) dense. Issue streams are ordered by need; the
        # xs pool (bufs=16) naturally delays gen1's xk DMAs until the kT
        # phase frees gen0's xk tiles, keeping the rings clear at startup.
        gens[0].issue_xk()
        gens[0].issue_xv(nc.scalar)

        for c in range(NDIN):
            t = wpool.tile([128, 512], BF16, name=f"wq{c}", tag=f"wq{c}")
            nc.scalar.dma_start(t[:], wq[c * 128:(c + 1) * 128, :])
            wqc.append(t)
        for hp in range(NHP):
            t = wpool.tile([128, D], BF16, name=f"wo{hp}", tag=f"wo{hp}")
            nc.scalar.dma_start(t[:], wo[hp * 128:(hp + 1) * 128, :])
            woh2.append(t)
        # trigger the Exp act-table load before the first real softmax
        dume = const.tile([1, 16], F32, name="dume", tag="dume")
        nc.scalar.activation(dume[:], dum1[:], mybir.ActivationFunctionType.Exp,
                             scale=1.0)

        gens[1].issue_xk()
        pacer.add(gens[0].gen(), 96)
        pacer.drain()

        for sb in range(NSB):
            if sb < NSB - 1:
                pacer.add(gens[sb + 1].gen(), 96)
            else:
                for psb in range(3):
                    pacer.add(proj_gen(psb), 32)
            nck = 4 * sb + 4
            pacer.slots_left = NHP * nck
            ats = [None] * NHP
            for hp in range(NHP):
                pvA = pvps.tile([128, SB], F32, name="pvA", tag="pv")
                pvB = pvps.tile([128, SB], F32, name="pvB", tag="pv")
                pend = deque()
                # chunks processed in pairs: both pairs' score matmuls form
                # one 64-contraction PE stretch, then masks+PVs+fills form a
                # 128-contraction stretch (fewer PE tile-config switches)
                for j in range(nck // 2):
                    s0 = emit_score_mms(sb, hp, 2 * j)
                    s1 = emit_score_mms(sb, hp, 2 * j + 1)
                    pend.append(emit_mask_exp(s0))
                    pend.append(emit_mask_exp(s1))
                    if len(pend) == 4:
                        emit_pv(hp, pend.popleft(), pvA, pvB, nck)
                        emit_pv(hp, pend.popleft(), pvA, pvB, nck)
                    pacer.slot()
                    pacer.slot()
                while pend:
                    emit_pv(hp, pend.popleft(), pvA, pvB, nck)
                    pacer.draw(1)
                ats[hp] = emit_normalize(sb, hp, pvA, pvB)
                if sb == NSB - 1 and hp == 1:
                    at_of[3] = ats  # list mutated in place as hp 2,3 finish
                    pacer.add(proj3_part1(), 16, front=True)
            at_of[sb] = ats
            pacer.drain()

        proj3_part2()

    nc.compile()
    return nc


_NC = None
_LAST_IN_MAPS = None


def _get_nc():
    global _NC
    if _NC is None:
        _NC = build_program()
    return _NC


def _build_msk():
    m = np.concatenate(
        [np.triu(np.full((128, 128), NEG, np.float32), 1),
         np.eye(128, dtype=np.float32)],
        axis=1,
    )
    return np.ascontiguousarray(m.astype(ml_dtypes.bfloat16))


def kernel(query, key, value, mask, Wq, bq, Wk, bk, Wv, bv, Wo, bo):
    query = np.asarray(query, np.float32)
    key = np.asarray(key, np.float32)
    value = np.asarray(value, np.float32)
    Wq = np.asarray(Wq, np.float32)
    Wk = np.asarray(Wk, np.float32)
    Wv = np.asarray(Wv, np.float32)
    Wo = np.asarray(Wo, np.float32)
    bq = np.asarray(bq, np.float32)
    bk = np.asarray(bk, np.float32)
    bv = np.asarray(bv, np.float32)
    bo = np.asarray(bo, np.float32)

    nc = _get_nc()

    B = query.shape[0]
    bf = ml_dtypes.bfloat16
    xq_t = [np.ascontiguousarray(query[b].T.astype(bf)) for b in range(B)]
    xk_t = [np.ascontiguousarray(key[b].T.astype(bf)) for b in range(B)]
    xv_t = [np.ascontiguousarray(value[b].T.astype(bf)) for b in range(B)]
    msk_np = _build_msk()

    in_maps = []
    for c in range(8):
        b, hg = c // 2, c % 2
        sl = slice(hg * 512, (hg + 1) * 512)
        in_maps.append({
            "xq_t": xq_t[b],
            "xk_t": xk_t[b],
            "xv_t": xv_t[b],
            "wq_t": np.ascontiguousarray(Wq[sl, :].T.astype(bf)),
            "wkv_t": np.ascontiguousarray(
                np.concatenate([Wk[sl, :].T, Wv[sl, :].T], axis=1).astype(bf)),
            "wo_t": np.ascontiguousarray(Wo[:, sl].T.astype(bf)),
            "bq_s": np.ascontiguousarray(bq[sl, None]),
            "bk_s": np.ascontiguousarray(bk[sl, None]),
            "bv_r": np.ascontiguousarray(bv[None, sl]),
            "msk": msk_np,
        })

    global _LAST_IN_MAPS
    _LAST_IN_MAPS = in_maps
    res = run_bass_kernel_spmd(nc, in_maps, core_ids=list(range(8)))
    out = np.empty((B, S, D), np.float32)
    for b in range(B):
        out[b] = res.results[2 * b]["y"] + res.results[2 * b + 1]["y"]
    out += bo[None, None, :]
    return out
